# revision 1
# baseline (speedup 1.0000x reference)
"""Trainium2 Bass kernel for nn_Net_53386443489635 (spiral-conv GNN).

Data-parallel over nodes on 8 NeuronCores:
  - each core computes a 12500-node shard (padded to 12800) of every layer
  - h0/h1/h2 are AllGathered between conv layers so each core can gather
    neighbor features from the full node table
  - neighbor gathers use indirect DMA (128 rows per call, int32 indices)
  - gathered tiles are PE-transposed to feature-major, conv matmuls run with
    stationary weight chunks accumulating in PSUM
  - ELU is composed as max(x,0) + exp(min(x,0)) - 1
  - conv3 -> fc1 -> fc2 stay feature-major on-chip; fc2 emits node-major
    logits and log_softmax runs along the free axis
"""

import sys

for _p in ("/opt/trn_rl_repo", "/root/.axon_site/_ro/trn_rl_repo"):
    if _p not in sys.path:
        sys.path.append(_p)

import numpy as np

import concourse.bass as bass
import concourse.bacc as bacc
import concourse.mybir as mybir
import concourse.tile as tile
from concourse.bass import IndirectOffsetOnAxis
from concourse.bass_utils import run_bass_kernel_spmd

F32 = mybir.dt.float32
I32 = mybir.dt.int32
ALU = mybir.AluOpType
AF = mybir.ActivationFunctionType
AX = mybir.AxisListType

N_CORES = 8
P = 128
SEQ = 16
IN_C = 3
NUM_CLASSES = 12
GROUP = 512          # nodes per group (4 sub-tiles of 128)
NSUB = GROUP // P    # 4


class Cfg:
    def __init__(self, n_nodes=100000, shard=12500, shard_pad=12800):
        assert shard_pad % GROUP == 0
        self.n_nodes = n_nodes
        self.shard = shard
        self.shard_pad = shard_pad
        self.n_groups = shard_pad // GROUP
        self.table = N_CORES * shard_pad


FULL = Cfg()
MINI = Cfg(n_nodes=4000, shard=500, shard_pad=512)

# (S*C_in, C_in, C_out) per conv layer
CONV_DIMS = [(SEQ * 16, 16, 32), (SEQ * 32, 32, 64), (SEQ * 64, 64, 128)]


def _emit_elu(nc, sb, psum_in, bias_col, c_out, width, tag):
    """elu(psum_in + bias) -> returns SBUF tile [c_out, width]."""
    y = sb.tile([c_out, width], F32, name=f"y_{tag}", tag="elu_y")
    nc.vector.tensor_scalar(out=y[:], in0=psum_in, scalar1=bias_col,
                            scalar2=None, op0=ALU.add)
    r = sb.tile([c_out, width], F32, name=f"r_{tag}", tag="elu_r")
    nc.vector.tensor_scalar(out=r[:], in0=y[:], scalar1=0.0, scalar2=None,
                            op0=ALU.max)
    n = sb.tile([c_out, width], F32, name=f"n_{tag}", tag="elu_n")
    nc.vector.tensor_scalar(out=n[:], in0=y[:], scalar1=0.0, scalar2=None,
                            op0=ALU.min)
    e = sb.tile([c_out, width], F32, name=f"e_{tag}", tag="elu_e")
    nc.scalar.activation(out=e[:], in_=n[:], func=AF.Exp)
    h = sb.tile([c_out, width], F32, name=f"h_{tag}", tag=f"h_{tag[:2]}")
    nc.vector.scalar_tensor_tensor(out=h[:], in0=e[:], scalar=-1.0, in1=r[:],
                                   op0=ALU.add, op1=ALU.add)
    return h


def build(cfg: Cfg):
    nc = bacc.Bacc("TRN2", target_bir_lowering=False, debug=False,
                   enable_asserts=True, num_devices=N_CORES)

    sp = cfg.shard_pad
    ng = cfg.n_groups

    # ---- I/O ----
    x_in = nc.dram_tensor("x_in", [sp, IN_C], F32, kind="ExternalInput")
    idx_in = nc.dram_tensor("idx_in", [P, ng * 64], I32, kind="ExternalInput")
    ident_in = nc.dram_tensor("ident_in", [P, P], F32, kind="ExternalInput")
    fc0_w_in = nc.dram_tensor("fc0_w_in", [IN_C, 16], F32, kind="ExternalInput")
    b0_in = nc.dram_tensor("b0_in", [16, 1], F32, kind="ExternalInput")
    w_ins, b_ins = [], []
    for li, (sc, ci, co) in enumerate(CONV_DIMS):
        w_ins.append(nc.dram_tensor(f"w{li + 1}_in", [P, (sc // P) * co], F32,
                                    kind="ExternalInput"))
        b_ins.append(nc.dram_tensor(f"b{li + 1}_in", [co, 1], F32,
                                    kind="ExternalInput"))
    fc1_w_in = nc.dram_tensor("fc1_w_in", [P, 256], F32, kind="ExternalInput")
    fc1_b_in = nc.dram_tensor("fc1_b_in", [P, 2], F32, kind="ExternalInput")
    fc2_w_in = nc.dram_tensor("fc2_w_in", [P, 2 * NUM_CLASSES], F32,
                              kind="ExternalInput")
    fc2_b_in = nc.dram_tensor("fc2_b_in", [P, NSUB * NUM_CLASSES], F32,
                              kind="ExternalInput")
    out_dram = nc.dram_tensor("out", [sp, NUM_CLASSES], F32,
                              kind="ExternalOutput")

    # ---- internal DRAM ----
    h_shard = [nc.dram_tensor(f"h{i}_shard", [sp, c], F32)
               for i, c in ((0, 16), (1, 32), (2, 64))]
    h_full = [nc.dram_tensor(f"h{i}_full", [cfg.table, c], F32,
                             addr_space="Shared")
              for i, c in ((0, 16), (1, 32), (2, 64))]

    rg = [list(range(N_CORES))]

    with tile.TileContext(nc) as tc:
        with (
            tc.tile_pool(name="sbuf", bufs=2) as sb,
            tc.tile_pool(name="wpool", bufs=1) as wp,
            tc.tile_pool(name="psum", bufs=2, space="PSUM") as pp,
        ):
            # ---- resident tiles ----
            idx_sb = wp.tile([P, ng * 64], I32, name="idx_sb")
            nc.sync.dma_start(out=idx_sb[:], in_=idx_in[:])
            ident = wp.tile([P, P], F32, name="ident")
            nc.sync.dma_start(out=ident[:], in_=ident_in[:])
            fc0_w = wp.tile([IN_C, 16], F32, name="fc0_w")
            nc.sync.dma_start(out=fc0_w[:], in_=fc0_w_in[:])
            b0 = wp.tile([16, 1], F32, name="b0")
            nc.sync.dma_start(out=b0[:], in_=b0_in[:])
            conv_w, conv_b = [], []
            for li, (sc, ci, co) in enumerate(CONV_DIMS):
                w = wp.tile([P, (sc // P) * co], F32, name=f"w{li + 1}")
                nc.sync.dma_start(out=w[:], in_=w_ins[li][:])
                b = wp.tile([co, 1], F32, name=f"bb{li + 1}")
                nc.sync.dma_start(out=b[:], in_=b_ins[li][:])
                conv_w.append(w)
                conv_b.append(b)
            fc1_w = wp.tile([P, 256], F32, name="fc1_w")
            nc.sync.dma_start(out=fc1_w[:], in_=fc1_w_in[:])
            fc1_b = wp.tile([P, 2], F32, name="fc1_b")
            nc.sync.dma_start(out=fc1_b[:], in_=fc1_b_in[:])
            fc2_w = wp.tile([P, 2 * NUM_CLASSES], F32, name="fc2_w")
            nc.sync.dma_start(out=fc2_w[:], in_=fc2_w_in[:])
            fc2_b = wp.tile([P, NSUB * NUM_CLASSES], F32, name="fc2_b")
            nc.sync.dma_start(out=fc2_b[:], in_=fc2_b_in[:])

            # ---- fc0: x -> h0_shard (node-major) ----
            for g in range(ng):
                gsl = slice(g * GROUP, (g + 1) * GROUP)
                x_sb = sb.tile([P, NSUB * IN_C], F32, name=f"x_{g}", tag="x")
                nc.sync.dma_start(
                    out=x_sb[:].rearrange("p (s c) -> p s c", c=IN_C),
                    in_=x_in[gsl, :].rearrange("(s p) c -> p s c", p=P))
                xt_ps = pp.tile([IN_C, GROUP], F32, name=f"xtp_{g}", tag="psA")
                for s in range(NSUB):
                    nc.tensor.transpose(
                        out=xt_ps[:, s * P:(s + 1) * P],
                        in_=x_sb[:, s * IN_C:(s + 1) * IN_C],
                        identity=ident[:])
                xt_sb = sb.tile([IN_C, GROUP], F32, name=f"xts_{g}", tag="xts")
                nc.vector.tensor_copy(out=xt_sb[:], in_=xt_ps[:])
                h0t_ps = pp.tile([16, GROUP], F32, name=f"h0tp_{g}", tag="psB")
                nc.tensor.matmul(out=h0t_ps[:], lhsT=fc0_w[:], rhs=xt_sb[:],
                                 start=True, stop=True, skip_group_check=True)
                h0t = _emit_elu(nc, sb, h0t_ps[:], b0[:, 0:1], 16, GROUP,
                                f"f0_{g}")
                bt_ps = pp.tile([P, NSUB * 16], F32, name=f"h0bt_{g}",
                                tag="psC")
                for s in range(NSUB):
                    nc.tensor.transpose(
                        out=bt_ps[:, s * 16:(s + 1) * 16],
                        in_=h0t[:, s * P:(s + 1) * P],
                        identity=ident[:16, :16])
                bt_sb = sb.tile([P, NSUB * 16], F32, name=f"h0bs_{g}",
                                tag="h0bs")
                nc.vector.tensor_copy(out=bt_sb[:], in_=bt_ps[:])
                nc.sync.dma_start(
                    out=h_shard[0][gsl, :].rearrange("(s p) c -> p s c", p=P),
                    in_=bt_sb[:].rearrange("p (s c) -> p s c", c=16))

            nc.gpsimd.collective_compute(
                "AllGather", ALU.bypass, replica_groups=rg,
                ins=[h_shard[0][:]], outs=[h_full[0][:]])

            # ---- fc1 + fc2 + log_softmax tail (runs per conv3 group) ----
            def emit_tail(g, h3t):
                gsl = slice(g * GROUP, (g + 1) * GROUP)
                h4t = []
                for half in range(2):
                    h4_ps = pp.tile([P, GROUP], F32, name=f"h4p_{g}_{half}",
                                    tag="psC")
                    nc.tensor.matmul(
                        out=h4_ps[:], lhsT=fc1_w[:, half * P:(half + 1) * P],
                        rhs=h3t[:], start=True, stop=True,
                        skip_group_check=True)
                    h4t.append(_emit_elu(nc, sb, h4_ps[:],
                                         fc1_b[:, half:half + 1], P, GROUP,
                                         f"f{half}1_{g}"))
                nclw = NUM_CLASSES
                lg_ps = pp.tile([P, NSUB * nclw], F32, name=f"lg_{g}",
                                tag="psD", bufs=1)
                for s in range(NSUB):
                    for half in range(2):
                        nc.tensor.matmul(
                            out=lg_ps[:, s * nclw:(s + 1) * nclw],
                            lhsT=h4t[half][:, s * P:(s + 1) * P],
                            rhs=fc2_w[:, half * nclw:(half + 1) * nclw],
                            start=(half == 0), stop=(half == 1),
                            skip_group_check=True)
                lg_sb = sb.tile([P, NSUB * nclw], F32, name=f"lgs_{g}",
                                tag="lgs")
                nc.vector.tensor_tensor(out=lg_sb[:], in0=lg_ps[:],
                                        in1=fc2_b[:], op=ALU.add)
                lg3 = lg_sb[:].rearrange("p (s c) -> p s c", c=nclw)
                mx = sb.tile([P, NSUB], F32, name=f"mx_{g}", tag="mx")
                nc.vector.tensor_reduce(
                    out=mx[:].rearrange("p (s o) -> p s o", o=1),
                    in_=lg3, axis=AX.X, op=ALU.max)
                sh_sb = sb.tile([P, NSUB * nclw], F32, name=f"sh_{g}", tag="sh")
                nc.vector.tensor_tensor(
                    out=sh_sb[:].rearrange("p (s c) -> p s c", c=nclw),
                    in0=lg3,
                    in1=mx[:].rearrange("p (s o) -> p s o", o=1).to_broadcast(
                        [P, NSUB, nclw]),
                    op=ALU.subtract)
                ex_sb = sb.tile([P, NSUB * nclw], F32, name=f"ex_{g}", tag="ex")
                nc.scalar.activation(out=ex_sb[:], in_=sh_sb[:], func=AF.Exp)
                sm = sb.tile([P, NSUB], F32, name=f"sm_{g}", tag="sm")
                nc.vector.tensor_reduce(
                    out=sm[:].rearrange("p (s o) -> p s o", o=1),
                    in_=ex_sb[:].rearrange("p (s c) -> p s c", c=nclw),
                    axis=AX.X, op=ALU.add)
                ls = sb.tile([P, NSUB], F32, name=f"ls_{g}", tag="ls")
                nc.scalar.activation(out=ls[:], in_=sm[:], func=AF.Ln)
                res = sb.tile([P, NSUB * nclw], F32, name=f"res_{g}", tag="res")
                nc.vector.tensor_tensor(
                    out=res[:].rearrange("p (s c) -> p s c", c=nclw),
                    in0=sh_sb[:].rearrange("p (s c) -> p s c", c=nclw),
                    in1=ls[:].rearrange("p (s o) -> p s o", o=1).to_broadcast(
                        [P, NSUB, nclw]),
                    op=ALU.subtract)
                nc.sync.dma_start(
                    out=out_dram[gsl, :].rearrange("(s p) c -> p s c", p=P),
                    in_=res[:].rearrange("p (s c) -> p s c", c=nclw))

            # ---- conv layers ----
            for li, (sc, ci, co) in enumerate(CONV_DIMS):
                kch = sc // P
                src = h_full[li]
                for g in range(ng):
                    gsl = slice(g * GROUP, (g + 1) * GROUP)
                    gt = sb.tile([P, NSUB * sc], F32, name=f"G{li}_{g}",
                                 tag=f"G{li}")
                    for slot in range(64):
                        nc.gpsimd.indirect_dma_start(
                            out=gt[:, slot * ci:(slot + 1) * ci],
                            out_offset=None,
                            in_=src[:],
                            in_offset=IndirectOffsetOnAxis(
                                ap=idx_sb[:, g * 64 + slot:g * 64 + slot + 1],
                                axis=0))
                    ot_ps = pp.tile([co, GROUP], F32, name=f"ot{li}_{g}",
                                    tag="psB")
                    for k in range(kch):
                        gtp = pp.tile([P, GROUP], F32, name=f"gtp{li}_{g}_{k}",
                                      tag="psA")
                        for s in range(NSUB):
                            nc.tensor.transpose(
                                out=gtp[:, s * P:(s + 1) * P],
                                in_=gt[:, s * sc + k * P:s * sc + (k + 1) * P],
                                identity=ident[:])
                        gts = sb.tile([P, GROUP], F32, name=f"gts{li}_{g}_{k}",
                                      tag=f"gts{li}")
                        nc.vector.tensor_copy(out=gts[:], in_=gtp[:])
                        nc.tensor.matmul(
                            out=ot_ps[:],
                            lhsT=conv_w[li][:, k * co:(k + 1) * co],
                            rhs=gts[:], start=(k == 0), stop=(k == kch - 1),
                            skip_group_check=True)
                    ht = _emit_elu(nc, sb, ot_ps[:], conv_b[li][:, 0:1], co,
                                   GROUP, f"c{li}_{g}")
                    if li < 2:
                        bt_ps = pp.tile([P, NSUB * co], F32,
                                        name=f"bt{li}_{g}", tag="psC")
                        for s in range(NSUB):
                            nc.tensor.transpose(
                                out=bt_ps[:, s * co:(s + 1) * co],
                                in_=ht[:, s * P:(s + 1) * P],
                                identity=ident[:co, :co])
                        bt_sb = sb.tile([P, NSUB * co], F32,
                                        name=f"bts{li}_{g}", tag=f"bts{li}")
                        nc.vector.tensor_copy(out=bt_sb[:], in_=bt_ps[:])
                        nc.sync.dma_start(
                            out=h_shard[li + 1][gsl, :].rearrange(
                                "(s p) c -> p s c", p=P),
                            in_=bt_sb[:].rearrange("p (s c) -> p s c", c=co))
                    else:
                        emit_tail(g, ht)
                if li < 2:
                    nc.gpsimd.collective_compute(
                        "AllGather", ALU.bypass, replica_groups=rg,
                        ins=[h_shard[li + 1][:]], outs=[h_full[li + 1][:]])

    nc.compile()
    return nc


def _prep_inputs(cfg: Cfg, x, indices, fc0_w, fc0_b, w1, b1, w2, b2, w3, b3,
                 fc1_w, fc1_b, fc2_w, fc2_b):
    """Shard + rearrange host inputs into per-core in_maps."""
    x = np.asarray(x, np.float32)
    idx = np.asarray(indices, np.int64)
    # remap node ids into the padded table coordinates
    idx2 = ((idx // cfg.shard) * cfg.shard_pad + idx % cfg.shard).astype(np.int32)

    def conv_w_prep(w):
        # w [C_out, S*C] -> lhsT chunks [128, kch*C_out]
        w = np.asarray(w, np.float32)
        co, sc = w.shape
        kch = sc // P
        return np.ascontiguousarray(
            w.T.reshape(kch, P, co).transpose(1, 0, 2).reshape(P, kch * co))

    common = {
        "ident_in": np.eye(P, dtype=np.float32),
        "fc0_w_in": np.ascontiguousarray(np.asarray(fc0_w, np.float32).T),
        "b0_in": np.asarray(fc0_b, np.float32).reshape(16, 1),
        "w1_in": conv_w_prep(w1), "b1_in": np.asarray(b1, np.float32).reshape(-1, 1),
        "w2_in": conv_w_prep(w2), "b2_in": np.asarray(b2, np.float32).reshape(-1, 1),
        "w3_in": conv_w_prep(w3), "b3_in": np.asarray(b3, np.float32).reshape(-1, 1),
        "fc1_w_in": np.ascontiguousarray(np.asarray(fc1_w, np.float32).T),
        "fc1_b_in": np.ascontiguousarray(
            np.asarray(fc1_b, np.float32).reshape(2, P).T),
        "fc2_w_in": np.ascontiguousarray(
            np.asarray(fc2_w, np.float32).T.reshape(2, P, NUM_CLASSES)
            .transpose(1, 0, 2).reshape(P, 2 * NUM_CLASSES)),
        "fc2_b_in": np.tile(np.asarray(fc2_b, np.float32).reshape(1, NUM_CLASSES),
                            (P, NSUB)),
    }

    in_maps = []
    for c in range(N_CORES):
        lo = c * cfg.shard
        xs = np.zeros((cfg.shard_pad, IN_C), np.float32)
        xs[:cfg.shard] = x[lo:lo + cfg.shard]
        ic = np.zeros((cfg.shard_pad, SEQ), np.int32)
        ic[:cfg.shard] = idx2[lo:lo + cfg.shard]
        idx_tiles = np.ascontiguousarray(
            ic.reshape(cfg.n_groups, NSUB, P, SEQ).transpose(2, 0, 1, 3)
            .reshape(P, cfg.n_groups * 64))
        in_maps.append({"x_in": xs, "idx_in": idx_tiles, **common})
    return in_maps


_NC_CACHE = {}


def _get_nc(cfg: Cfg):
    key = cfg.shard_pad
    if key not in _NC_CACHE:
        _NC_CACHE[key] = build(cfg)
    return _NC_CACHE[key]


def kernel(**inputs) -> np.ndarray:
    cfg = FULL
    nc = _get_nc(cfg)
    in_maps = _prep_inputs(cfg, **inputs)
    res = run_bass_kernel_spmd(nc, in_maps, list(range(N_CORES)))
    out = np.concatenate(
        [res.results[c]["out"][:cfg.shard] for c in range(N_CORES)], axis=0)
    return out.astype(np.float32)



# revision 14
# speedup vs baseline: 1.1154x; 1.1154x over previous
"""Trainium2 Bass kernel for nn_Net_53386443489635 (spiral-conv GNN).

Data-parallel over nodes on 8 NeuronCores, v2 (batched quad gathers):
  - each core computes a 12500-node shard (padded to 12800) of every layer
  - h0/h1/h2 feature tables are bf16, stored quad-packed in DRAM (4 nodes
    per 256B/256B/512B row) and AllGathered between conv layers
  - neighbor gathers run as ONE dma_gather per 512-node group (8192 int16
    quad indices, positional layout chosen host-side so the output lands
    node-major), then a 4-way predicated select resolves the within-quad
    offset using host-precomputed masks
  - selected bf16 tiles are PE-transposed to feature-major, conv matmuls run
    bf16 x bf16 with fp32 PSUM accumulation
  - ELU = relu(y) + exp(y - relu(y)) - 1 split across DVE and ACT
  - conv3 -> fc1 -> fc2 run fp32r; fc2 emits node-major logits and
    log_softmax runs along the free axis
"""

import sys

for _p in ("/opt/trn_rl_repo", "/root/.axon_site/_ro/trn_rl_repo"):
    if _p not in sys.path:
        sys.path.append(_p)

import numpy as np
import ml_dtypes

import concourse.bass as bass
import concourse.bacc as bacc
import concourse.mybir as mybir
import concourse.tile as tile
from concourse.bass_utils import run_bass_kernel_spmd

F32 = mybir.dt.float32
F32R = mybir.dt.float32r
BF16 = mybir.dt.bfloat16
I16 = mybir.dt.int16
U8 = mybir.dt.uint8
ALU = mybir.AluOpType
AF = mybir.ActivationFunctionType
AX = mybir.AxisListType

N_CORES = 8
P = 128
SEQ = 16
IN_C = 3
NUM_CLASSES = 12
GROUP = 512          # nodes per group (4 sub-tiles of 128)
NSUB = GROUP // P    # 4
NIDX = GROUP * SEQ   # 8192 gathers per group

# conv layer l: (S*C_in, C_in, C_out, table pitch in bf16 elems per node)
CONV_DIMS = [(SEQ * 16, 16, 32, 32), (SEQ * 32, 32, 64, 32),
             (SEQ * 64, 64, 128, 64)]
PITCHES = [32, 32, 64]   # h0, h1, h2 table pitches (>= real ch, 64B-mult/4)


class Cfg:
    def __init__(self, n_nodes=100000, shard=12500, shard_pad=12800):
        assert shard_pad % GROUP == 0
        self.n_nodes = n_nodes
        self.shard = shard
        self.shard_pad = shard_pad
        self.n_groups = shard_pad // GROUP
        self.table = N_CORES * shard_pad
        self.n_quads = self.table // 4


FULL = Cfg()
MINI = Cfg(n_nodes=4000, shard=500, shard_pad=512)


def _emit_elu(nc, sb, psum_in, bias_col, c_out, width, tag, out_dtype=F32):
    """elu(psum_in + bias) -> SBUF tile [c_out, width]. 3 DVE + 2 ACT ops."""
    y = sb.tile([c_out, width], F32, name=f"y_{tag}", tag="elu_y")
    nc.vector.tensor_scalar(out=y[:], in0=psum_in, scalar1=bias_col,
                            scalar2=None, op0=ALU.add)
    r = sb.tile([c_out, width], F32, name=f"r_{tag}", tag="elu_r")
    nc.scalar.activation(out=r[:], in_=y[:], func=AF.Relu)
    n = sb.tile([c_out, width], F32, name=f"n_{tag}", tag="elu_n")
    nc.vector.tensor_tensor(out=n[:], in0=y[:], in1=r[:], op=ALU.subtract)
    e = sb.tile([c_out, width], F32, name=f"e_{tag}", tag="elu_e")
    nc.scalar.activation(out=e[:], in_=n[:], func=AF.Exp)
    h = sb.tile([c_out, width], out_dtype, name=f"h_{tag}", tag=f"h_{tag[:2]}")
    nc.vector.scalar_tensor_tensor(out=h[:], in0=e[:], scalar=-1.0, in1=r[:],
                                   op0=ALU.add, op1=ALU.add)
    return h


def build(cfg: Cfg):
    nc = bacc.Bacc("TRN2", target_bir_lowering=False, debug=False,
                   enable_asserts=True, num_devices=N_CORES)

    sp = cfg.shard_pad
    ng = cfg.n_groups

    # ---- I/O ----
    x_in = nc.dram_tensor("x_in", [sp, IN_C], F32, kind="ExternalInput")
    gidx_in = nc.dram_tensor("gidx_in", [P, ng * GROUP], I16,
                             kind="ExternalInput")
    msk_in = nc.dram_tensor("msk_in", [P, ng * 3 * 64], U8,
                            kind="ExternalInput")
    ident_in = nc.dram_tensor("ident_in", [P, P], F32, kind="ExternalInput")
    fc0_w_in = nc.dram_tensor("fc0_w_in", [IN_C, 16], F32,
                              kind="ExternalInput")
    b0_in = nc.dram_tensor("b0_in", [16, 1], F32, kind="ExternalInput")
    w_ins, b_ins = [], []
    for li, (sc, ci, co, _pt) in enumerate(CONV_DIMS):
        w_ins.append(nc.dram_tensor(f"w{li + 1}_in", [P, (sc // P) * co],
                                    BF16, kind="ExternalInput"))
        b_ins.append(nc.dram_tensor(f"b{li + 1}_in", [co, 1], F32,
                                    kind="ExternalInput"))
    fc1_w_in = nc.dram_tensor("fc1_w_in", [P, 256], F32, kind="ExternalInput")
    fc1_b_in = nc.dram_tensor("fc1_b_in", [P, 2], F32, kind="ExternalInput")
    fc2_w_in = nc.dram_tensor("fc2_w_in", [P, 2 * NUM_CLASSES], F32,
                              kind="ExternalInput")
    fc2_b_in = nc.dram_tensor("fc2_b_in", [P, NSUB * NUM_CLASSES], F32,
                              kind="ExternalInput")
    out_dram = nc.dram_tensor("out", [sp, NUM_CLASSES], F32,
                              kind="ExternalOutput")

    # ---- internal DRAM: bf16 quad-packed tables ----
    h_shard = [nc.dram_tensor(f"h{i}_shard", [sp, PITCHES[i]], BF16)
               for i in range(3)]
    h_full = [nc.dram_tensor(f"h{i}_full", [cfg.n_quads, 4 * PITCHES[i]],
                             BF16, addr_space="Shared")
              for i in range(3)]

    rg = [list(range(N_CORES))]

    with tile.TileContext(nc) as tc:
        with (
            tc.tile_pool(name="sbuf", bufs=2) as sb,
            tc.tile_pool(name="wpool", bufs=1) as wp,
            tc.tile_pool(name="psum", bufs=2, space="PSUM") as pp,
        ):
            # ---- resident tiles ----
            gidx_sb = wp.tile([P, ng * GROUP], I16, name="gidx_sb")
            nc.sync.dma_start(out=gidx_sb[:], in_=gidx_in[:])
            msk_sb = wp.tile([P, ng * 3 * 64], U8, name="msk_sb")
            nc.sync.dma_start(out=msk_sb[:], in_=msk_in[:])
            ident = wp.tile([P, P], F32, name="ident")
            nc.sync.dma_start(out=ident[:], in_=ident_in[:])
            identb = wp.tile([P, P], BF16, name="identb")
            nc.vector.tensor_copy(out=identb[:], in_=ident[:])

            def load_r(name, shape, src):
                """DMA fp32 -> SBUF, round once into an F32R tile."""
                t = wp.tile(shape, F32, name=f"{name}_raw")
                nc.sync.dma_start(out=t[:], in_=src[:])
                tr = wp.tile(shape, F32R, name=name)
                nc.vector.tensor_copy(out=tr[:], in_=t[:])
                return tr

            fc0_w = load_r("fc0_w", [IN_C, 16], fc0_w_in)
            b0 = wp.tile([16, 1], F32, name="b0")
            nc.sync.dma_start(out=b0[:], in_=b0_in[:])
            conv_w, conv_b = [], []
            for li, (sc, ci, co, _pt) in enumerate(CONV_DIMS):
                w = wp.tile([P, (sc // P) * co], BF16, name=f"w{li + 1}")
                nc.sync.dma_start(out=w[:], in_=w_ins[li][:])
                conv_w.append(w)
                b = wp.tile([co, 1], F32, name=f"bb{li + 1}")
                nc.sync.dma_start(out=b[:], in_=b_ins[li][:])
                conv_b.append(b)
            fc1_w = load_r("fc1_w", [P, 256], fc1_w_in)
            fc1_b = wp.tile([P, 2], F32, name="fc1_b")
            nc.sync.dma_start(out=fc1_b[:], in_=fc1_b_in[:])
            fc2_w = load_r("fc2_w", [P, 2 * NUM_CLASSES], fc2_w_in)
            fc2_b = wp.tile([P, NSUB * NUM_CLASSES], F32, name="fc2_b")
            nc.sync.dma_start(out=fc2_b[:], in_=fc2_b_in[:])

            # ---- fc0: x -> h0_shard (node-major bf16) ----
            for g in range(ng):
                gsl = slice(g * GROUP, (g + 1) * GROUP)
                x_sb = sb.tile([P, NSUB * IN_C], F32, name=f"x_{g}", tag="x")
                nc.sync.dma_start(
                    out=x_sb[:].rearrange("p (s c) -> p s c", c=IN_C),
                    in_=x_in[gsl, :].rearrange("(s p) c -> p s c", p=P))
                xt_ps = pp.tile([IN_C, GROUP], F32, name=f"xtp_{g}", tag="psA")
                for s in range(NSUB):
                    nc.tensor.transpose(
                        out=xt_ps[:, s * P:(s + 1) * P],
                        in_=x_sb[:, s * IN_C:(s + 1) * IN_C],
                        identity=ident[:])
                xt_sb = sb.tile([IN_C, GROUP], F32R, name=f"xts_{g}",
                                tag="xts")
                nc.vector.tensor_copy(out=xt_sb[:], in_=xt_ps[:])
                h0t_ps = pp.tile([16, GROUP], F32, name=f"h0tp_{g}", tag="psB")
                nc.tensor.matmul(out=h0t_ps[:], lhsT=fc0_w[:], rhs=xt_sb[:],
                                 start=True, stop=True, skip_group_check=True)
                h0t = _emit_elu(nc, sb, h0t_ps[:], b0[:, 0:1], 16, GROUP,
                                f"f0_{g}", out_dtype=BF16)
                bt_ps = pp.tile([P, NSUB * 16], BF16, name=f"h0bt_{g}",
                                tag="psC")
                for s in range(NSUB):
                    nc.tensor.transpose(
                        out=bt_ps[:, s * 16:(s + 1) * 16],
                        in_=h0t[:, s * P:(s + 1) * P],
                        identity=identb[:16, :16])
                bt_sb = sb.tile([P, NSUB * 16], BF16, name=f"h0bs_{g}",
                                tag="bts")
                nc.vector.tensor_copy(out=bt_sb[:], in_=bt_ps[:])
                nc.sync.dma_start(
                    out=h_shard[0][gsl, 0:16].rearrange("(s p) c -> p s c",
                                                        p=P),
                    in_=bt_sb[:].rearrange("p (s c) -> p s c", c=16))

            nc.gpsimd.collective_compute(
                "AllGather", ALU.bypass, replica_groups=rg,
                ins=[h_shard[0][:]], outs=[h_full[0][:]])

            # ---- fc1 + fc2 + log_softmax tail (runs per conv3 group) ----
            def emit_tail(g, h3t):
                gsl = slice(g * GROUP, (g + 1) * GROUP)
                h4t = []
                for half in range(2):
                    h4_ps = pp.tile([P, GROUP], F32, name=f"h4p_{g}_{half}",
                                    tag="psB")
                    nc.tensor.matmul(
                        out=h4_ps[:], lhsT=fc1_w[:, half * P:(half + 1) * P],
                        rhs=h3t[:], start=True, stop=True,
                        skip_group_check=True)
                    h4t.append(_emit_elu(nc, sb, h4_ps[:],
                                         fc1_b[:, half:half + 1], P, GROUP,
                                         f"f{half}1_{g}", out_dtype=F32R))
                nclw = NUM_CLASSES
                lg_ps = pp.tile([P, NSUB * nclw], F32, name=f"lg_{g}",
                                tag="psD", bufs=1)
                for s in range(NSUB):
                    for half in range(2):
                        nc.tensor.matmul(
                            out=lg_ps[:, s * nclw:(s + 1) * nclw],
                            lhsT=h4t[half][:, s * P:(s + 1) * P],
                            rhs=fc2_w[:, half * nclw:(half + 1) * nclw],
                            start=(half == 0), stop=(half == 1),
                            skip_group_check=True)
                lg_sb = sb.tile([P, NSUB * nclw], F32, name=f"lgs_{g}",
                                tag="lgs")
                nc.vector.tensor_tensor(out=lg_sb[:], in0=lg_ps[:],
                                        in1=fc2_b[:], op=ALU.add)
                lg3 = lg_sb[:].rearrange("p (s c) -> p s c", c=nclw)
                mx = sb.tile([P, NSUB], F32, name=f"mx_{g}", tag="mx")
                nc.vector.tensor_reduce(
                    out=mx[:].rearrange("p (s o) -> p s o", o=1),
                    in_=lg3, axis=AX.X, op=ALU.max)
                sh_sb = sb.tile([P, NSUB * nclw], F32, name=f"sh_{g}",
                                tag="sh")
                nc.vector.tensor_tensor(
                    out=sh_sb[:].rearrange("p (s c) -> p s c", c=nclw),
                    in0=lg3,
                    in1=mx[:].rearrange("p (s o) -> p s o", o=1).to_broadcast(
                        [P, NSUB, nclw]),
                    op=ALU.subtract)
                ex_sb = sb.tile([P, NSUB * nclw], F32, name=f"ex_{g}",
                                tag="ex")
                nc.scalar.activation(out=ex_sb[:], in_=sh_sb[:], func=AF.Exp)
                sm = sb.tile([P, NSUB], F32, name=f"sm_{g}", tag="sm")
                nc.vector.tensor_reduce(
                    out=sm[:].rearrange("p (s o) -> p s o", o=1),
                    in_=ex_sb[:].rearrange("p (s c) -> p s c", c=nclw),
                    axis=AX.X, op=ALU.add)
                ls = sb.tile([P, NSUB], F32, name=f"ls_{g}", tag="ls")
                nc.scalar.activation(out=ls[:], in_=sm[:], func=AF.Ln)
                res = sb.tile([P, NSUB * nclw], F32, name=f"res_{g}",
                              tag="res")
                nc.vector.tensor_tensor(
                    out=res[:].rearrange("p (s c) -> p s c", c=nclw),
                    in0=sh_sb[:].rearrange("p (s c) -> p s c", c=nclw),
                    in1=ls[:].rearrange("p (s o) -> p s o", o=1).to_broadcast(
                        [P, NSUB, nclw]),
                    op=ALU.subtract)
                nc.sync.dma_start(
                    out=out_dram[gsl, :].rearrange("(s p) c -> p s c", p=P),
                    in_=res[:].rearrange("p (s c) -> p s c", c=nclw))

            # ---- conv layers ----
            for li, (sc, ci, co, pitch) in enumerate(CONV_DIMS):
                kch = sc // P
                qelem = 4 * pitch
                src = h_full[li]
                for g in range(ng):
                    gsl = slice(g * GROUP, (g + 1) * GROUP)
                    # one batched quad gather for the whole group
                    qt = sb.tile([P, 64 * qelem], BF16, name=f"qt{li}_{g}",
                                 tag="qt")
                    nc.gpsimd.dma_gather(
                        out_ap=qt[:].rearrange("p (s e) -> p s e", e=qelem),
                        in_ap=src[:],
                        idxs_ap=gidx_sb[:, g * GROUP:(g + 1) * GROUP],
                        num_idxs=NIDX,
                        num_idxs_reg=NIDX,
                        elem_size=qelem,
                        single_packet=False)
                    qv = qt[:].rearrange("p (s o c) -> p s o c", o=4, c=pitch)
                    # 4-way within-quad select into compact node-major tile
                    cp = sb.tile([P, 64 * ci], BF16, name=f"cp{li}_{g}",
                                 tag="cp")
                    cpv = cp[:].rearrange("p (s c) -> p s c", c=ci)
                    nc.vector.tensor_copy(out=cpv, in_=qv[:, :, 0, 0:ci])
                    for t in (1, 2, 3):
                        mbase = g * 192 + (t - 1) * 64
                        m = msk_sb[:, mbase:mbase + 64].rearrange(
                            "p (s u) -> p s u", u=1).to_broadcast([P, 64, ci])
                        nc.vector.copy_predicated(out=cpv, mask=m,
                                                  data=qv[:, :, t, 0:ci])
                    # feature-major transposes + matmul
                    ot_ps = pp.tile([co, GROUP], F32, name=f"ot{li}_{g}",
                                    tag="psB")
                    for k in range(kch):
                        gtp = pp.tile([P, GROUP], BF16, name=f"gtp{li}_{g}_{k}",
                                      tag="psA")
                        for s in range(NSUB):
                            nc.tensor.transpose(
                                out=gtp[:, s * P:(s + 1) * P],
                                in_=cp[:, s * sc + k * P:s * sc + (k + 1) * P],
                                identity=identb[:])
                        gts = sb.tile([P, GROUP], BF16, name=f"gts{li}_{g}_{k}",
                                      tag="gts")
                        nc.vector.tensor_copy(out=gts[:], in_=gtp[:])
                        nc.tensor.matmul(
                            out=ot_ps[:],
                            lhsT=conv_w[li][:, k * co:(k + 1) * co],
                            rhs=gts[:], start=(k == 0), stop=(k == kch - 1),
                            skip_group_check=True)
                    if li < 2:
                        ht = _emit_elu(nc, sb, ot_ps[:], conv_b[li][:, 0:1],
                                       co, GROUP, f"c{li}_{g}",
                                       out_dtype=BF16)
                        bt_ps = pp.tile([P, NSUB * co], BF16,
                                        name=f"bt{li}_{g}", tag="psC")
                        for s in range(NSUB):
                            nc.tensor.transpose(
                                out=bt_ps[:, s * co:(s + 1) * co],
                                in_=ht[:, s * P:(s + 1) * P],
                                identity=identb[:co, :co])
                        bt_sb = sb.tile([P, NSUB * co], BF16,
                                        name=f"bts{li}_{g}", tag="bts")
                        nc.vector.tensor_copy(out=bt_sb[:], in_=bt_ps[:])
                        nc.sync.dma_start(
                            out=h_shard[li + 1][gsl, 0:co].rearrange(
                                "(s p) c -> p s c", p=P),
                            in_=bt_sb[:].rearrange("p (s c) -> p s c", c=co))
                    else:
                        ht = _emit_elu(nc, sb, ot_ps[:], conv_b[li][:, 0:1],
                                       co, GROUP, f"c{li}_{g}",
                                       out_dtype=F32R)
                        emit_tail(g, ht)
                if li < 2:
                    nc.gpsimd.collective_compute(
                        "AllGather", ALU.bypass, replica_groups=rg,
                        ins=[h_shard[li + 1][:]], outs=[h_full[li + 1][:]])

    nc.compile()
    return nc


def _prep_inputs(cfg: Cfg, x, indices, fc0_w, fc0_b, w1, b1, w2, b2, w3, b3,
                 fc1_w, fc1_b, fc2_w, fc2_b):
    """Shard + rearrange host inputs into per-core in_maps."""
    x = np.asarray(x, np.float32)
    idx = np.asarray(indices, np.int64)
    # node id -> padded table row
    rows = ((idx // cfg.shard) * cfg.shard_pad + idx % cfg.shard).astype(
        np.int32)
    quads = (rows // 4).astype(np.int16)
    offs = (rows % 4).astype(np.uint8)

    def conv_w_prep(w):
        # w [C_out, S*C] -> lhsT chunks [128, kch*C_out], bf16
        w = np.asarray(w, np.float32)
        co, sc = w.shape
        kch = sc // P
        return np.ascontiguousarray(
            w.T.reshape(kch, P, co).transpose(1, 0, 2).reshape(P, kch * co)
        ).astype(ml_dtypes.bfloat16)

    common = {
        "ident_in": np.eye(P, dtype=np.float32),
        "fc0_w_in": np.ascontiguousarray(np.asarray(fc0_w, np.float32).T),
        "b0_in": np.asarray(fc0_b, np.float32).reshape(16, 1),
        "w1_in": conv_w_prep(w1),
        "b1_in": np.asarray(b1, np.float32).reshape(-1, 1),
        "w2_in": conv_w_prep(w2),
        "b2_in": np.asarray(b2, np.float32).reshape(-1, 1),
        "w3_in": conv_w_prep(w3),
        "b3_in": np.asarray(b3, np.float32).reshape(-1, 1),
        "fc1_w_in": np.ascontiguousarray(np.asarray(fc1_w, np.float32).T),
        "fc1_b_in": np.ascontiguousarray(
            np.asarray(fc1_b, np.float32).reshape(2, P).T),
        "fc2_w_in": np.ascontiguousarray(
            np.asarray(fc2_w, np.float32).T.reshape(2, P, NUM_CLASSES)
            .transpose(1, 0, 2).reshape(P, 2 * NUM_CLASSES)),
        "fc2_b_in": np.tile(
            np.asarray(fc2_b, np.float32).reshape(1, NUM_CLASSES), (P, NSUB)),
    }

    ng = cfg.n_groups
    in_maps = []
    for c in range(N_CORES):
        lo = c * cfg.shard
        xs = np.zeros((cfg.shard_pad, IN_C), np.float32)
        xs[:cfg.shard] = x[lo:lo + cfg.shard]
        qc = np.zeros((cfg.shard_pad, SEQ), np.int16)
        qc[:cfg.shard] = quads[lo:lo + cfg.shard]
        oc = np.zeros((cfg.shard_pad, SEQ), np.uint8)
        oc[:cfg.shard] = offs[lo:lo + cfg.shard]

        # gather list position i = slot*128 + p, slot = s*16 + j,
        # node = g*512 + s*128 + p
        # qg[g, s, p, j] -> list[g, (s,j), p]
        qg = qc.reshape(ng, NSUB, P, SEQ)
        lists = qg.transpose(0, 1, 3, 2).reshape(ng, NIDX)   # [g, i]
        # wrapped [16, NIDX//16]: wrapped[i%16, i//16] = list[i], then
        # replicated 8x across the 128 partitions (one copy per Q7 core)
        wrapped = lists.reshape(ng, NIDX // 16, 16).transpose(0, 2, 1)
        gidx = np.tile(wrapped, (1, 8, 1))            # [ng, 128, 512]
        gidx = np.ascontiguousarray(
            gidx.transpose(1, 0, 2).reshape(P, ng * GROUP))

        og = oc.reshape(ng, NSUB, P, SEQ).transpose(0, 1, 3, 2) \
            .reshape(ng, 64, P)                               # [g, slot, p]
        msk = np.zeros((P, ng * 3 * 64), np.uint8)
        for t in (1, 2, 3):
            sel = (og == t).astype(np.uint8)                  # [g, slot, p]
            for g in range(ng):
                msk[:, g * 192 + (t - 1) * 64:g * 192 + t * 64] = \
                    sel[g].T                                  # [p, slot]
        in_maps.append({"x_in": xs, "gidx_in": gidx, "msk_in": msk, **common})
    return in_maps


_NC_CACHE = {}


def _get_nc(cfg: Cfg):
    key = cfg.shard_pad
    if key not in _NC_CACHE:
        _NC_CACHE[key] = build(cfg)
    return _NC_CACHE[key]


def kernel(**inputs) -> np.ndarray:
    cfg = FULL
    nc = _get_nc(cfg)
    in_maps = _prep_inputs(cfg, **inputs)
    res = run_bass_kernel_spmd(nc, in_maps, list(range(N_CORES)))
    out = np.concatenate(
        [res.results[c]["out"][:cfg.shard] for c in range(N_CORES)], axis=0)
    return out.astype(np.float32)


# revision 15
# speedup vs baseline: 1.2355x; 1.1076x over previous
"""Trainium2 Bass kernel for nn_Net_53386443489635 (spiral-conv GNN).

Data-parallel over nodes on 8 NeuronCores, v2 (batched quad gathers):
  - each core computes a 12500-node shard (padded to 12800) of every layer
  - h0/h1/h2 feature tables are bf16, stored quad-packed in DRAM (4 nodes
    per 256B/256B/512B row) and AllGathered between conv layers
  - neighbor gathers run as ONE dma_gather per 512-node group (8192 int16
    quad indices, positional layout chosen host-side so the output lands
    node-major), then a 4-way predicated select resolves the within-quad
    offset using host-precomputed masks
  - selected bf16 tiles are PE-transposed to feature-major, conv matmuls run
    bf16 x bf16 with fp32 PSUM accumulation
  - ELU = relu(y) + exp(y - relu(y)) - 1 split across DVE and ACT
  - conv3 -> fc1 -> fc2 run fp32r; fc2 emits node-major logits and
    log_softmax runs along the free axis
"""

import sys

for _p in ("/opt/trn_rl_repo", "/root/.axon_site/_ro/trn_rl_repo"):
    if _p not in sys.path:
        sys.path.append(_p)

import numpy as np
import ml_dtypes

import concourse.bass as bass
import concourse.bacc as bacc
import concourse.mybir as mybir
import concourse.tile as tile
from concourse.bass_utils import run_bass_kernel_spmd

F32 = mybir.dt.float32
F32R = mybir.dt.float32r
BF16 = mybir.dt.bfloat16
I16 = mybir.dt.int16
U8 = mybir.dt.uint8
ALU = mybir.AluOpType
AF = mybir.ActivationFunctionType
AX = mybir.AxisListType

N_CORES = 8
P = 128
SEQ = 16
IN_C = 3
NUM_CLASSES = 12
GROUP = 512          # nodes per group (4 sub-tiles of 128)
NSUB = GROUP // P    # 4
NIDX = GROUP * SEQ   # 8192 gathers per group

# conv layer l: (S*C_in, C_in, C_out, table pitch in bf16 elems per node)
CONV_DIMS = [(SEQ * 16, 16, 32, 32), (SEQ * 32, 32, 64, 32),
             (SEQ * 64, 64, 128, 64)]
PITCHES = [32, 32, 64]   # h0, h1, h2 table pitches (>= real ch, 64B-mult/4)


class Cfg:
    def __init__(self, n_nodes=100000, shard=12500, shard_pad=12800):
        assert shard_pad % GROUP == 0
        self.n_nodes = n_nodes
        self.shard = shard
        self.shard_pad = shard_pad
        self.n_groups = shard_pad // GROUP
        self.table = N_CORES * shard_pad
        self.n_quads = self.table // 4


FULL = Cfg()
MINI = Cfg(n_nodes=4000, shard=500, shard_pad=512)


def _emit_elu(nc, sb, psum_in, bias_col, c_out, width, tag, out_dtype=F32):
    """elu(psum_in + bias) -> SBUF tile [c_out, width]. 3 DVE + 2 ACT ops."""
    y = sb.tile([c_out, width], F32, name=f"y_{tag}", tag="elu_y")
    nc.vector.tensor_scalar(out=y[:], in0=psum_in, scalar1=bias_col,
                            scalar2=None, op0=ALU.add)
    r = sb.tile([c_out, width], F32, name=f"r_{tag}", tag="elu_r")
    nc.scalar.activation(out=r[:], in_=y[:], func=AF.Relu)
    n = sb.tile([c_out, width], F32, name=f"n_{tag}", tag="elu_n")
    nc.vector.tensor_tensor(out=n[:], in0=y[:], in1=r[:], op=ALU.subtract)
    e = sb.tile([c_out, width], F32, name=f"e_{tag}", tag="elu_e")
    nc.scalar.activation(out=e[:], in_=n[:], func=AF.Exp)
    h = sb.tile([c_out, width], out_dtype, name=f"h_{tag}", tag=f"h_{tag[:2]}")
    nc.vector.scalar_tensor_tensor(out=h[:], in0=e[:], scalar=-1.0, in1=r[:],
                                   op0=ALU.add, op1=ALU.add)
    return h


def build(cfg: Cfg):
    nc = bacc.Bacc("TRN2", target_bir_lowering=False, debug=False,
                   enable_asserts=True, num_devices=N_CORES,
                   num_swdge_queues=4)

    sp = cfg.shard_pad
    ng = cfg.n_groups

    # ---- I/O ----
    x_in = nc.dram_tensor("x_in", [sp, IN_C], F32, kind="ExternalInput")
    gidx_in = nc.dram_tensor("gidx_in", [P, ng * GROUP], I16,
                             kind="ExternalInput")
    msk_in = nc.dram_tensor("msk_in", [P, ng * 3 * 64], U8,
                            kind="ExternalInput")
    ident_in = nc.dram_tensor("ident_in", [P, P], F32, kind="ExternalInput")
    fc0_w_in = nc.dram_tensor("fc0_w_in", [IN_C, 16], F32,
                              kind="ExternalInput")
    b0_in = nc.dram_tensor("b0_in", [16, 1], F32, kind="ExternalInput")
    w_ins, b_ins = [], []
    for li, (sc, ci, co, _pt) in enumerate(CONV_DIMS):
        w_ins.append(nc.dram_tensor(f"w{li + 1}_in", [P, (sc // P) * co],
                                    BF16, kind="ExternalInput"))
        b_ins.append(nc.dram_tensor(f"b{li + 1}_in", [co, 1], F32,
                                    kind="ExternalInput"))
    fc1_w_in = nc.dram_tensor("fc1_w_in", [P, 256], F32, kind="ExternalInput")
    fc1_b_in = nc.dram_tensor("fc1_b_in", [P, 2], F32, kind="ExternalInput")
    fc2_w_in = nc.dram_tensor("fc2_w_in", [P, 2 * NUM_CLASSES], F32,
                              kind="ExternalInput")
    fc2_b_in = nc.dram_tensor("fc2_b_in", [P, NSUB * NUM_CLASSES], F32,
                              kind="ExternalInput")
    out_dram = nc.dram_tensor("out", [sp, NUM_CLASSES], F32,
                              kind="ExternalOutput")

    # ---- internal DRAM: bf16 quad-packed tables ----
    h_shard = [nc.dram_tensor(f"h{i}_shard", [sp, PITCHES[i]], BF16)
               for i in range(3)]
    h_full = [nc.dram_tensor(f"h{i}_full", [cfg.n_quads, 4 * PITCHES[i]],
                             BF16, addr_space="Shared")
              for i in range(3)]

    rg = [list(range(N_CORES))]

    with tile.TileContext(nc) as tc:
        with (
            tc.tile_pool(name="sbuf", bufs=2) as sb,
            tc.tile_pool(name="wpool", bufs=1) as wp,
            tc.tile_pool(name="psum", bufs=2, space="PSUM") as pp,
        ):
            # ---- resident tiles ----
            gidx_sb = wp.tile([P, ng * GROUP], I16, name="gidx_sb")
            nc.sync.dma_start(out=gidx_sb[:], in_=gidx_in[:])
            msk_sb = wp.tile([P, ng * 3 * 64], U8, name="msk_sb")
            nc.sync.dma_start(out=msk_sb[:], in_=msk_in[:])
            ident = wp.tile([P, P], F32, name="ident")
            nc.sync.dma_start(out=ident[:], in_=ident_in[:])
            identb = wp.tile([P, P], BF16, name="identb")
            nc.vector.tensor_copy(out=identb[:], in_=ident[:])

            def load_r(name, shape, src):
                """DMA fp32 -> SBUF, round once into an F32R tile."""
                t = wp.tile(shape, F32, name=f"{name}_raw")
                nc.sync.dma_start(out=t[:], in_=src[:])
                tr = wp.tile(shape, F32R, name=name)
                nc.vector.tensor_copy(out=tr[:], in_=t[:])
                return tr

            fc0_w = load_r("fc0_w", [IN_C, 16], fc0_w_in)
            b0 = wp.tile([16, 1], F32, name="b0")
            nc.sync.dma_start(out=b0[:], in_=b0_in[:])
            conv_w, conv_b = [], []
            for li, (sc, ci, co, _pt) in enumerate(CONV_DIMS):
                w = wp.tile([P, (sc // P) * co], BF16, name=f"w{li + 1}")
                nc.sync.dma_start(out=w[:], in_=w_ins[li][:])
                conv_w.append(w)
                b = wp.tile([co, 1], F32, name=f"bb{li + 1}")
                nc.sync.dma_start(out=b[:], in_=b_ins[li][:])
                conv_b.append(b)
            fc1_w = load_r("fc1_w", [P, 256], fc1_w_in)
            fc1_b = wp.tile([P, 2], F32, name="fc1_b")
            nc.sync.dma_start(out=fc1_b[:], in_=fc1_b_in[:])
            fc2_w = load_r("fc2_w", [P, 2 * NUM_CLASSES], fc2_w_in)
            fc2_b = wp.tile([P, NSUB * NUM_CLASSES], F32, name="fc2_b")
            nc.sync.dma_start(out=fc2_b[:], in_=fc2_b_in[:])

            # ---- fc0: x -> h0_shard (node-major bf16) ----
            for g in range(ng):
                gsl = slice(g * GROUP, (g + 1) * GROUP)
                x_sb = sb.tile([P, NSUB * IN_C], F32, name=f"x_{g}", tag="x")
                nc.sync.dma_start(
                    out=x_sb[:].rearrange("p (s c) -> p s c", c=IN_C),
                    in_=x_in[gsl, :].rearrange("(s p) c -> p s c", p=P))
                xt_ps = pp.tile([IN_C, GROUP], F32, name=f"xtp_{g}", tag="psA")
                for s in range(NSUB):
                    nc.tensor.transpose(
                        out=xt_ps[:, s * P:(s + 1) * P],
                        in_=x_sb[:, s * IN_C:(s + 1) * IN_C],
                        identity=ident[:])
                xt_sb = sb.tile([IN_C, GROUP], F32R, name=f"xts_{g}",
                                tag="xts")
                nc.vector.tensor_copy(out=xt_sb[:], in_=xt_ps[:])
                h0t_ps = pp.tile([16, GROUP], F32, name=f"h0tp_{g}", tag="psB")
                nc.tensor.matmul(out=h0t_ps[:], lhsT=fc0_w[:], rhs=xt_sb[:],
                                 start=True, stop=True, skip_group_check=True)
                h0t = _emit_elu(nc, sb, h0t_ps[:], b0[:, 0:1], 16, GROUP,
                                f"f0_{g}", out_dtype=BF16)
                bt_ps = pp.tile([P, NSUB * 16], BF16, name=f"h0bt_{g}",
                                tag="psC")
                for s in range(NSUB):
                    nc.tensor.transpose(
                        out=bt_ps[:, s * 16:(s + 1) * 16],
                        in_=h0t[:, s * P:(s + 1) * P],
                        identity=identb[:16, :16])
                bt_sb = sb.tile([P, NSUB * 16], BF16, name=f"h0bs_{g}",
                                tag="bts")
                nc.vector.tensor_copy(out=bt_sb[:], in_=bt_ps[:])
                nc.sync.dma_start(
                    out=h_shard[0][gsl, 0:16].rearrange("(s p) c -> p s c",
                                                        p=P),
                    in_=bt_sb[:].rearrange("p (s c) -> p s c", c=16))

            nc.gpsimd.collective_compute(
                "AllGather", ALU.bypass, replica_groups=rg,
                ins=[h_shard[0][:]], outs=[h_full[0][:]])

            # ---- fc1 + fc2 + log_softmax tail (runs per conv3 group) ----
            def emit_tail(g, h3t):
                gsl = slice(g * GROUP, (g + 1) * GROUP)
                h4t = []
                for half in range(2):
                    h4_ps = pp.tile([P, GROUP], F32, name=f"h4p_{g}_{half}",
                                    tag="psB")
                    nc.tensor.matmul(
                        out=h4_ps[:], lhsT=fc1_w[:, half * P:(half + 1) * P],
                        rhs=h3t[:], start=True, stop=True,
                        skip_group_check=True)
                    h4t.append(_emit_elu(nc, sb, h4_ps[:],
                                         fc1_b[:, half:half + 1], P, GROUP,
                                         f"f{half}1_{g}", out_dtype=F32R))
                nclw = NUM_CLASSES
                lg_ps = pp.tile([P, NSUB * nclw], F32, name=f"lg_{g}",
                                tag="psD", bufs=1)
                for s in range(NSUB):
                    for half in range(2):
                        nc.tensor.matmul(
                            out=lg_ps[:, s * nclw:(s + 1) * nclw],
                            lhsT=h4t[half][:, s * P:(s + 1) * P],
                            rhs=fc2_w[:, half * nclw:(half + 1) * nclw],
                            start=(half == 0), stop=(half == 1),
                            skip_group_check=True)
                lg_sb = sb.tile([P, NSUB * nclw], F32, name=f"lgs_{g}",
                                tag="lgs")
                nc.vector.tensor_tensor(out=lg_sb[:], in0=lg_ps[:],
                                        in1=fc2_b[:], op=ALU.add)
                lg3 = lg_sb[:].rearrange("p (s c) -> p s c", c=nclw)
                mx = sb.tile([P, NSUB], F32, name=f"mx_{g}", tag="mx")
                nc.vector.tensor_reduce(
                    out=mx[:].rearrange("p (s o) -> p s o", o=1),
                    in_=lg3, axis=AX.X, op=ALU.max)
                sh_sb = sb.tile([P, NSUB * nclw], F32, name=f"sh_{g}",
                                tag="sh")
                nc.vector.tensor_tensor(
                    out=sh_sb[:].rearrange("p (s c) -> p s c", c=nclw),
                    in0=lg3,
                    in1=mx[:].rearrange("p (s o) -> p s o", o=1).to_broadcast(
                        [P, NSUB, nclw]),
                    op=ALU.subtract)
                ex_sb = sb.tile([P, NSUB * nclw], F32, name=f"ex_{g}",
                                tag="ex")
                nc.scalar.activation(out=ex_sb[:], in_=sh_sb[:], func=AF.Exp)
                sm = sb.tile([P, NSUB], F32, name=f"sm_{g}", tag="sm")
                nc.vector.tensor_reduce(
                    out=sm[:].rearrange("p (s o) -> p s o", o=1),
                    in_=ex_sb[:].rearrange("p (s c) -> p s c", c=nclw),
                    axis=AX.X, op=ALU.add)
                ls = sb.tile([P, NSUB], F32, name=f"ls_{g}", tag="ls")
                nc.scalar.activation(out=ls[:], in_=sm[:], func=AF.Ln)
                res = sb.tile([P, NSUB * nclw], F32, name=f"res_{g}",
                              tag="res")
                nc.vector.tensor_tensor(
                    out=res[:].rearrange("p (s c) -> p s c", c=nclw),
                    in0=sh_sb[:].rearrange("p (s c) -> p s c", c=nclw),
                    in1=ls[:].rearrange("p (s o) -> p s o", o=1).to_broadcast(
                        [P, NSUB, nclw]),
                    op=ALU.subtract)
                nc.sync.dma_start(
                    out=out_dram[gsl, :].rearrange("(s p) c -> p s c", p=P),
                    in_=res[:].rearrange("p (s c) -> p s c", c=nclw))

            # ---- conv layers ----
            for li, (sc, ci, co, pitch) in enumerate(CONV_DIMS):
                kch = sc // P
                qelem = 4 * pitch
                src = h_full[li]
                for g in range(ng):
                    gsl = slice(g * GROUP, (g + 1) * GROUP)
                    # one batched quad gather for the whole group
                    qt = sb.tile([P, 64 * qelem], BF16, name=f"qt{li}_{g}",
                                 tag="qt")
                    nc.gpsimd.dma_gather(
                        out_ap=qt[:].rearrange("p (s e) -> p s e", e=qelem),
                        in_ap=src[:],
                        idxs_ap=gidx_sb[:, g * GROUP:(g + 1) * GROUP],
                        num_idxs=NIDX,
                        num_idxs_reg=NIDX,
                        elem_size=qelem,
                        single_packet=False,
                        queue_num=g % 4)
                    qv = qt[:].rearrange("p (s o c) -> p s o c", o=4, c=pitch)
                    # 4-way within-quad select into compact node-major tile
                    cp = sb.tile([P, 64 * ci], BF16, name=f"cp{li}_{g}",
                                 tag="cp")
                    cpv = cp[:].rearrange("p (s c) -> p s c", c=ci)
                    nc.vector.tensor_copy(out=cpv, in_=qv[:, :, 0, 0:ci])
                    for t in (1, 2, 3):
                        mbase = g * 192 + (t - 1) * 64
                        m = msk_sb[:, mbase:mbase + 64].rearrange(
                            "p (s u) -> p s u", u=1).to_broadcast([P, 64, ci])
                        nc.vector.copy_predicated(out=cpv, mask=m,
                                                  data=qv[:, :, t, 0:ci])
                    # feature-major transposes + matmul
                    ot_ps = pp.tile([co, GROUP], F32, name=f"ot{li}_{g}",
                                    tag="psB")
                    for k in range(kch):
                        gtp = pp.tile([P, GROUP], BF16, name=f"gtp{li}_{g}_{k}",
                                      tag="psA")
                        for s in range(NSUB):
                            nc.tensor.transpose(
                                out=gtp[:, s * P:(s + 1) * P],
                                in_=cp[:, s * sc + k * P:s * sc + (k + 1) * P],
                                identity=identb[:])
                        gts = sb.tile([P, GROUP], BF16, name=f"gts{li}_{g}_{k}",
                                      tag="gts")
                        nc.vector.tensor_copy(out=gts[:], in_=gtp[:])
                        nc.tensor.matmul(
                            out=ot_ps[:],
                            lhsT=conv_w[li][:, k * co:(k + 1) * co],
                            rhs=gts[:], start=(k == 0), stop=(k == kch - 1),
                            skip_group_check=True)
                    if li < 2:
                        ht = _emit_elu(nc, sb, ot_ps[:], conv_b[li][:, 0:1],
                                       co, GROUP, f"c{li}_{g}",
                                       out_dtype=BF16)
                        bt_ps = pp.tile([P, NSUB * co], BF16,
                                        name=f"bt{li}_{g}", tag="psC")
                        for s in range(NSUB):
                            nc.tensor.transpose(
                                out=bt_ps[:, s * co:(s + 1) * co],
                                in_=ht[:, s * P:(s + 1) * P],
                                identity=identb[:co, :co])
                        bt_sb = sb.tile([P, NSUB * co], BF16,
                                        name=f"bts{li}_{g}", tag="bts")
                        nc.vector.tensor_copy(out=bt_sb[:], in_=bt_ps[:])
                        nc.sync.dma_start(
                            out=h_shard[li + 1][gsl, 0:co].rearrange(
                                "(s p) c -> p s c", p=P),
                            in_=bt_sb[:].rearrange("p (s c) -> p s c", c=co))
                    else:
                        ht = _emit_elu(nc, sb, ot_ps[:], conv_b[li][:, 0:1],
                                       co, GROUP, f"c{li}_{g}",
                                       out_dtype=F32R)
                        emit_tail(g, ht)
                if li < 2:
                    nc.gpsimd.collective_compute(
                        "AllGather", ALU.bypass, replica_groups=rg,
                        ins=[h_shard[li + 1][:]], outs=[h_full[li + 1][:]])

    nc.compile()
    return nc


def _prep_inputs(cfg: Cfg, x, indices, fc0_w, fc0_b, w1, b1, w2, b2, w3, b3,
                 fc1_w, fc1_b, fc2_w, fc2_b):
    """Shard + rearrange host inputs into per-core in_maps."""
    x = np.asarray(x, np.float32)
    idx = np.asarray(indices, np.int64)
    # node id -> padded table row
    rows = ((idx // cfg.shard) * cfg.shard_pad + idx % cfg.shard).astype(
        np.int32)
    quads = (rows // 4).astype(np.int16)
    offs = (rows % 4).astype(np.uint8)

    def conv_w_prep(w):
        # w [C_out, S*C] -> lhsT chunks [128, kch*C_out], bf16
        w = np.asarray(w, np.float32)
        co, sc = w.shape
        kch = sc // P
        return np.ascontiguousarray(
            w.T.reshape(kch, P, co).transpose(1, 0, 2).reshape(P, kch * co)
        ).astype(ml_dtypes.bfloat16)

    common = {
        "ident_in": np.eye(P, dtype=np.float32),
        "fc0_w_in": np.ascontiguousarray(np.asarray(fc0_w, np.float32).T),
        "b0_in": np.asarray(fc0_b, np.float32).reshape(16, 1),
        "w1_in": conv_w_prep(w1),
        "b1_in": np.asarray(b1, np.float32).reshape(-1, 1),
        "w2_in": conv_w_prep(w2),
        "b2_in": np.asarray(b2, np.float32).reshape(-1, 1),
        "w3_in": conv_w_prep(w3),
        "b3_in": np.asarray(b3, np.float32).reshape(-1, 1),
        "fc1_w_in": np.ascontiguousarray(np.asarray(fc1_w, np.float32).T),
        "fc1_b_in": np.ascontiguousarray(
            np.asarray(fc1_b, np.float32).reshape(2, P).T),
        "fc2_w_in": np.ascontiguousarray(
            np.asarray(fc2_w, np.float32).T.reshape(2, P, NUM_CLASSES)
            .transpose(1, 0, 2).reshape(P, 2 * NUM_CLASSES)),
        "fc2_b_in": np.tile(
            np.asarray(fc2_b, np.float32).reshape(1, NUM_CLASSES), (P, NSUB)),
    }

    ng = cfg.n_groups
    in_maps = []
    for c in range(N_CORES):
        lo = c * cfg.shard
        xs = np.zeros((cfg.shard_pad, IN_C), np.float32)
        xs[:cfg.shard] = x[lo:lo + cfg.shard]
        qc = np.zeros((cfg.shard_pad, SEQ), np.int16)
        qc[:cfg.shard] = quads[lo:lo + cfg.shard]
        oc = np.zeros((cfg.shard_pad, SEQ), np.uint8)
        oc[:cfg.shard] = offs[lo:lo + cfg.shard]

        # gather list position i = slot*128 + p, slot = s*16 + j,
        # node = g*512 + s*128 + p
        # qg[g, s, p, j] -> list[g, (s,j), p]
        qg = qc.reshape(ng, NSUB, P, SEQ)
        lists = qg.transpose(0, 1, 3, 2).reshape(ng, NIDX)   # [g, i]
        # wrapped [16, NIDX//16]: wrapped[i%16, i//16] = list[i], then
        # replicated 8x across the 128 partitions (one copy per Q7 core)
        wrapped = lists.reshape(ng, NIDX // 16, 16).transpose(0, 2, 1)
        gidx = np.tile(wrapped, (1, 8, 1))            # [ng, 128, 512]
        gidx = np.ascontiguousarray(
            gidx.transpose(1, 0, 2).reshape(P, ng * GROUP))

        og = oc.reshape(ng, NSUB, P, SEQ).transpose(0, 1, 3, 2) \
            .reshape(ng, 64, P)                               # [g, slot, p]
        msk = np.zeros((P, ng * 3 * 64), np.uint8)
        for t in (1, 2, 3):
            sel = (og == t).astype(np.uint8)                  # [g, slot, p]
            for g in range(ng):
                msk[:, g * 192 + (t - 1) * 64:g * 192 + t * 64] = \
                    sel[g].T                                  # [p, slot]
        in_maps.append({"x_in": xs, "gidx_in": gidx, "msk_in": msk, **common})
    return in_maps


_NC_CACHE = {}


def _get_nc(cfg: Cfg):
    key = cfg.shard_pad
    if key not in _NC_CACHE:
        _NC_CACHE[key] = build(cfg)
    return _NC_CACHE[key]


def kernel(**inputs) -> np.ndarray:
    cfg = FULL
    nc = _get_nc(cfg)
    in_maps = _prep_inputs(cfg, **inputs)
    res = run_bass_kernel_spmd(nc, in_maps, list(range(N_CORES)))
    out = np.concatenate(
        [res.results[c]["out"][:cfg.shard] for c in range(N_CORES)], axis=0)
    return out.astype(np.float32)


# revision 16
# speedup vs baseline: 2.6395x; 2.1365x over previous
"""Trainium2 Bass kernel for nn_Net_53386443489635 (spiral-conv GNN).

Data-parallel over nodes on 8 NeuronCores, v2 (batched quad gathers):
  - each core computes a 12500-node shard (padded to 12800) of every layer
  - h0/h1/h2 feature tables are bf16, stored quad-packed in DRAM (4 nodes
    per 256B/256B/512B row) and AllGathered between conv layers
  - neighbor gathers run as ONE dma_gather per 512-node group (8192 int16
    quad indices, positional layout chosen host-side so the output lands
    node-major), then a 4-way predicated select resolves the within-quad
    offset using host-precomputed masks
  - selected bf16 tiles are PE-transposed to feature-major, conv matmuls run
    bf16 x bf16 with fp32 PSUM accumulation
  - ELU = relu(y) + exp(y - relu(y)) - 1 split across DVE and ACT
  - conv3 -> fc1 -> fc2 run fp32r; fc2 emits node-major logits and
    log_softmax runs along the free axis
"""

import sys

for _p in ("/opt/trn_rl_repo", "/root/.axon_site/_ro/trn_rl_repo"):
    if _p not in sys.path:
        sys.path.append(_p)

import numpy as np
import ml_dtypes

import concourse.bass as bass
import concourse.bacc as bacc
import concourse.mybir as mybir
import concourse.tile as tile
from concourse.bass_utils import run_bass_kernel_spmd

F32 = mybir.dt.float32
F32R = mybir.dt.float32r
BF16 = mybir.dt.bfloat16
I16 = mybir.dt.int16
U8 = mybir.dt.uint8
ALU = mybir.AluOpType
AF = mybir.ActivationFunctionType
AX = mybir.AxisListType

N_CORES = 8
P = 128
SEQ = 16
IN_C = 3
NUM_CLASSES = 12
GROUP = 512          # nodes per group (4 sub-tiles of 128)
NSUB = GROUP // P    # 4
NIDX = GROUP * SEQ   # 8192 gathers per group

# conv layer l: (S*C_in, C_in, C_out, table pitch in bf16 elems per node)
CONV_DIMS = [(SEQ * 16, 16, 32, 32), (SEQ * 32, 32, 64, 32),
             (SEQ * 64, 64, 128, 64)]
PITCHES = [32, 32, 64]   # h0, h1, h2 table pitches (>= real ch, 64B-mult/4)


class Cfg:
    def __init__(self, n_nodes=100000, shard=12500, shard_pad=12800):
        assert shard_pad % GROUP == 0
        self.n_nodes = n_nodes
        self.shard = shard
        self.shard_pad = shard_pad
        self.n_groups = shard_pad // GROUP
        self.table = N_CORES * shard_pad
        self.n_quads = self.table // 4


FULL = Cfg()
MINI = Cfg(n_nodes=4000, shard=500, shard_pad=512)


def _emit_elu(nc, sb, psum_in, bias_col, c_out, width, tag, out_dtype=F32):
    """elu(psum_in + bias) -> SBUF tile [c_out, width]. 3 DVE + 2 ACT ops."""
    y = sb.tile([c_out, width], F32, name=f"y_{tag}", tag="elu_y")
    nc.vector.tensor_scalar(out=y[:], in0=psum_in, scalar1=bias_col,
                            scalar2=None, op0=ALU.add)
    r = sb.tile([c_out, width], F32, name=f"r_{tag}", tag="elu_r")
    nc.scalar.activation(out=r[:], in_=y[:], func=AF.Relu)
    n = sb.tile([c_out, width], F32, name=f"n_{tag}", tag="elu_n")
    nc.vector.tensor_tensor(out=n[:], in0=y[:], in1=r[:], op=ALU.subtract)
    e = sb.tile([c_out, width], F32, name=f"e_{tag}", tag="elu_e")
    nc.scalar.activation(out=e[:], in_=n[:], func=AF.Exp)
    h = sb.tile([c_out, width], out_dtype, name=f"h_{tag}", tag=f"h_{tag[:2]}")
    nc.vector.scalar_tensor_tensor(out=h[:], in0=e[:], scalar=-1.0, in1=r[:],
                                   op0=ALU.add, op1=ALU.add)
    return h


def build(cfg: Cfg):
    nc = bacc.Bacc("TRN2", target_bir_lowering=False, debug=False,
                   enable_asserts=True, num_devices=N_CORES,
                   num_swdge_queues=4)

    sp = cfg.shard_pad
    ng = cfg.n_groups

    # ---- I/O ----
    x_in = nc.dram_tensor("x_in", [sp, IN_C], F32, kind="ExternalInput")
    gidx_in = nc.dram_tensor("gidx_in", [P, ng * GROUP], I16,
                             kind="ExternalInput")
    msk_in = nc.dram_tensor("msk_in", [P, ng * 3 * 64], U8,
                            kind="ExternalInput")
    ident_in = nc.dram_tensor("ident_in", [P, P], F32, kind="ExternalInput")
    fc0_w_in = nc.dram_tensor("fc0_w_in", [IN_C, 16], F32,
                              kind="ExternalInput")
    b0_in = nc.dram_tensor("b0_in", [16, 1], F32, kind="ExternalInput")
    w_ins, b_ins = [], []
    for li, (sc, ci, co, _pt) in enumerate(CONV_DIMS):
        w_ins.append(nc.dram_tensor(f"w{li + 1}_in", [P, (sc // P) * co],
                                    BF16, kind="ExternalInput"))
        b_ins.append(nc.dram_tensor(f"b{li + 1}_in", [co, 1], F32,
                                    kind="ExternalInput"))
    fc1_w_in = nc.dram_tensor("fc1_w_in", [P, 256], F32, kind="ExternalInput")
    fc1_b_in = nc.dram_tensor("fc1_b_in", [P, 2], F32, kind="ExternalInput")
    fc2_w_in = nc.dram_tensor("fc2_w_in", [P, 2 * NUM_CLASSES], F32,
                              kind="ExternalInput")
    fc2_b_in = nc.dram_tensor("fc2_b_in", [P, NSUB * NUM_CLASSES], F32,
                              kind="ExternalInput")
    out_dram = nc.dram_tensor("out", [sp, NUM_CLASSES], F32,
                              kind="ExternalOutput")

    # ---- internal DRAM: bf16 quad-packed tables ----
    h_shard = [nc.dram_tensor(f"h{i}_shard", [sp, PITCHES[i]], BF16)
               for i in range(3)]
    h_full = [nc.dram_tensor(f"h{i}_full", [cfg.n_quads, 4 * PITCHES[i]],
                             BF16, addr_space="Shared")
              for i in range(3)]

    rg = [list(range(N_CORES))]

    with tile.TileContext(nc) as tc:
        with (
            tc.tile_pool(name="sbuf", bufs=2) as sb,
            tc.tile_pool(name="wpool", bufs=1) as wp,
            tc.tile_pool(name="psum", bufs=2, space="PSUM") as pp,
        ):
            # ---- resident tiles ----
            gidx_sb = wp.tile([P, ng * GROUP], I16, name="gidx_sb")
            nc.sync.dma_start(out=gidx_sb[:], in_=gidx_in[:])
            msk_sb = wp.tile([P, ng * 3 * 64], U8, name="msk_sb")
            nc.sync.dma_start(out=msk_sb[:], in_=msk_in[:])
            ident = wp.tile([P, P], F32, name="ident")
            nc.sync.dma_start(out=ident[:], in_=ident_in[:])
            identb = wp.tile([P, P], BF16, name="identb")
            nc.vector.tensor_copy(out=identb[:], in_=ident[:])

            def load_r(name, shape, src):
                """DMA fp32 -> SBUF, round once into an F32R tile."""
                t = wp.tile(shape, F32, name=f"{name}_raw")
                nc.sync.dma_start(out=t[:], in_=src[:])
                tr = wp.tile(shape, F32R, name=name)
                nc.vector.tensor_copy(out=tr[:], in_=t[:])
                return tr

            fc0_w = load_r("fc0_w", [IN_C, 16], fc0_w_in)
            b0 = wp.tile([16, 1], F32, name="b0")
            nc.sync.dma_start(out=b0[:], in_=b0_in[:])
            conv_w, conv_b = [], []
            for li, (sc, ci, co, _pt) in enumerate(CONV_DIMS):
                w = wp.tile([P, (sc // P) * co], BF16, name=f"w{li + 1}")
                nc.sync.dma_start(out=w[:], in_=w_ins[li][:])
                conv_w.append(w)
                b = wp.tile([co, 1], F32, name=f"bb{li + 1}")
                nc.sync.dma_start(out=b[:], in_=b_ins[li][:])
                conv_b.append(b)
            fc1_w = load_r("fc1_w", [P, 256], fc1_w_in)
            fc1_b = wp.tile([P, 2], F32, name="fc1_b")
            nc.sync.dma_start(out=fc1_b[:], in_=fc1_b_in[:])
            fc2_w = load_r("fc2_w", [P, 2 * NUM_CLASSES], fc2_w_in)
            fc2_b = wp.tile([P, NSUB * NUM_CLASSES], F32, name="fc2_b")
            nc.sync.dma_start(out=fc2_b[:], in_=fc2_b_in[:])

            # ---- fc0: x -> h0_shard (node-major bf16) ----
            for g in range(ng):
                gsl = slice(g * GROUP, (g + 1) * GROUP)
                x_sb = sb.tile([P, NSUB * IN_C], F32, name=f"x_{g}", tag="x")
                nc.sync.dma_start(
                    out=x_sb[:].rearrange("p (s c) -> p s c", c=IN_C),
                    in_=x_in[gsl, :].rearrange("(s p) c -> p s c", p=P))
                xt_ps = pp.tile([IN_C, GROUP], F32, name=f"xtp_{g}", tag="psA")
                for s in range(NSUB):
                    nc.tensor.transpose(
                        out=xt_ps[:, s * P:(s + 1) * P],
                        in_=x_sb[:, s * IN_C:(s + 1) * IN_C],
                        identity=ident[:])
                xt_sb = sb.tile([IN_C, GROUP], F32R, name=f"xts_{g}",
                                tag="xts")
                nc.vector.tensor_copy(out=xt_sb[:], in_=xt_ps[:])
                h0t_ps = pp.tile([16, GROUP], F32, name=f"h0tp_{g}", tag="psB")
                nc.tensor.matmul(out=h0t_ps[:], lhsT=fc0_w[:], rhs=xt_sb[:],
                                 start=True, stop=True, skip_group_check=True)
                h0t = _emit_elu(nc, sb, h0t_ps[:], b0[:, 0:1], 16, GROUP,
                                f"f0_{g}", out_dtype=BF16)
                bt_ps = pp.tile([P, NSUB * 16], BF16, name=f"h0bt_{g}",
                                tag="psC")
                for s in range(NSUB):
                    nc.tensor.transpose(
                        out=bt_ps[:, s * 16:(s + 1) * 16],
                        in_=h0t[:, s * P:(s + 1) * P],
                        identity=identb[:16, :16])
                bt_sb = sb.tile([P, NSUB * 16], BF16, name=f"h0bs_{g}",
                                tag="bts")
                nc.vector.tensor_copy(out=bt_sb[:], in_=bt_ps[:])
                nc.sync.dma_start(
                    out=h_shard[0][gsl, 0:16].rearrange("(s p) c -> p s c",
                                                        p=P),
                    in_=bt_sb[:].rearrange("p (s c) -> p s c", c=16))

            nc.gpsimd.collective_compute(
                "AllGather", ALU.bypass, replica_groups=rg,
                ins=[h_shard[0][:]], outs=[h_full[0][:]])

            # ---- fc1 + fc2 + log_softmax tail (runs per conv3 group) ----
            def emit_tail(g, h3t):
                gsl = slice(g * GROUP, (g + 1) * GROUP)
                h4t = []
                for half in range(2):
                    h4_ps = pp.tile([P, GROUP], F32, name=f"h4p_{g}_{half}",
                                    tag="psB")
                    nc.tensor.matmul(
                        out=h4_ps[:], lhsT=fc1_w[:, half * P:(half + 1) * P],
                        rhs=h3t[:], start=True, stop=True,
                        skip_group_check=True)
                    h4t.append(_emit_elu(nc, sb, h4_ps[:],
                                         fc1_b[:, half:half + 1], P, GROUP,
                                         f"f{half}1_{g}", out_dtype=F32R))
                nclw = NUM_CLASSES
                lg_ps = pp.tile([P, NSUB * nclw], F32, name=f"lg_{g}",
                                tag="psD", bufs=1)
                for s in range(NSUB):
                    for half in range(2):
                        nc.tensor.matmul(
                            out=lg_ps[:, s * nclw:(s + 1) * nclw],
                            lhsT=h4t[half][:, s * P:(s + 1) * P],
                            rhs=fc2_w[:, half * nclw:(half + 1) * nclw],
                            start=(half == 0), stop=(half == 1),
                            skip_group_check=True)
                lg_sb = sb.tile([P, NSUB * nclw], F32, name=f"lgs_{g}",
                                tag="lgs")
                nc.vector.tensor_tensor(out=lg_sb[:], in0=lg_ps[:],
                                        in1=fc2_b[:], op=ALU.add)
                lg3 = lg_sb[:].rearrange("p (s c) -> p s c", c=nclw)
                mx = sb.tile([P, NSUB], F32, name=f"mx_{g}", tag="mx")
                nc.vector.tensor_reduce(
                    out=mx[:].rearrange("p (s o) -> p s o", o=1),
                    in_=lg3, axis=AX.X, op=ALU.max)
                sh_sb = sb.tile([P, NSUB * nclw], F32, name=f"sh_{g}",
                                tag="sh")
                nc.vector.tensor_tensor(
                    out=sh_sb[:].rearrange("p (s c) -> p s c", c=nclw),
                    in0=lg3,
                    in1=mx[:].rearrange("p (s o) -> p s o", o=1).to_broadcast(
                        [P, NSUB, nclw]),
                    op=ALU.subtract)
                ex_sb = sb.tile([P, NSUB * nclw], F32, name=f"ex_{g}",
                                tag="ex")
                nc.scalar.activation(out=ex_sb[:], in_=sh_sb[:], func=AF.Exp)
                sm = sb.tile([P, NSUB], F32, name=f"sm_{g}", tag="sm")
                nc.vector.tensor_reduce(
                    out=sm[:].rearrange("p (s o) -> p s o", o=1),
                    in_=ex_sb[:].rearrange("p (s c) -> p s c", c=nclw),
                    axis=AX.X, op=ALU.add)
                ls = sb.tile([P, NSUB], F32, name=f"ls_{g}", tag="ls")
                nc.scalar.activation(out=ls[:], in_=sm[:], func=AF.Ln)
                res = sb.tile([P, NSUB * nclw], F32, name=f"res_{g}",
                              tag="res")
                nc.vector.tensor_tensor(
                    out=res[:].rearrange("p (s c) -> p s c", c=nclw),
                    in0=sh_sb[:].rearrange("p (s c) -> p s c", c=nclw),
                    in1=ls[:].rearrange("p (s o) -> p s o", o=1).to_broadcast(
                        [P, NSUB, nclw]),
                    op=ALU.subtract)
                nc.sync.dma_start(
                    out=out_dram[gsl, :].rearrange("(s p) c -> p s c", p=P),
                    in_=res[:].rearrange("p (s c) -> p s c", c=nclw))

            # ---- conv layers ----
            for li, (sc, ci, co, pitch) in enumerate(CONV_DIMS):
                kch = sc // P
                qelem = 4 * pitch
                src = h_full[li]
                for g in range(ng):
                    gsl = slice(g * GROUP, (g + 1) * GROUP)
                    # one batched quad gather for the whole group
                    qt = sb.tile([P, 64 * qelem], BF16, name=f"qt{li}_{g}",
                                 tag="qt")
                    for q in range(4):
                        nc.gpsimd.dma_gather(
                            out_ap=qt[:, q * 16 * qelem:(q + 1) * 16 * qelem]
                            .rearrange("p (s e) -> p s e", e=qelem),
                            in_ap=src[:],
                            idxs_ap=gidx_sb[:, g * GROUP + q * 128:
                                            g * GROUP + (q + 1) * 128],
                            num_idxs=NIDX // 4,
                            num_idxs_reg=NIDX // 4,
                            elem_size=qelem,
                            single_packet=False,
                            queue_num=q)
                    qv = qt[:].rearrange("p (s o c) -> p s o c", o=4, c=pitch)
                    # 4-way within-quad select into compact node-major tile
                    cp = sb.tile([P, 64 * ci], BF16, name=f"cp{li}_{g}",
                                 tag="cp")
                    cpv = cp[:].rearrange("p (s c) -> p s c", c=ci)
                    nc.vector.tensor_copy(out=cpv, in_=qv[:, :, 0, 0:ci])
                    for t in (1, 2, 3):
                        mbase = g * 192 + (t - 1) * 64
                        m = msk_sb[:, mbase:mbase + 64].rearrange(
                            "p (s u) -> p s u", u=1).to_broadcast([P, 64, ci])
                        nc.vector.copy_predicated(out=cpv, mask=m,
                                                  data=qv[:, :, t, 0:ci])
                    # feature-major transposes + matmul
                    ot_ps = pp.tile([co, GROUP], F32, name=f"ot{li}_{g}",
                                    tag="psB")
                    for k in range(kch):
                        gtp = pp.tile([P, GROUP], BF16, name=f"gtp{li}_{g}_{k}",
                                      tag="psA")
                        for s in range(NSUB):
                            nc.tensor.transpose(
                                out=gtp[:, s * P:(s + 1) * P],
                                in_=cp[:, s * sc + k * P:s * sc + (k + 1) * P],
                                identity=identb[:])
                        gts = sb.tile([P, GROUP], BF16, name=f"gts{li}_{g}_{k}",
                                      tag="gts")
                        nc.vector.tensor_copy(out=gts[:], in_=gtp[:])
                        nc.tensor.matmul(
                            out=ot_ps[:],
                            lhsT=conv_w[li][:, k * co:(k + 1) * co],
                            rhs=gts[:], start=(k == 0), stop=(k == kch - 1),
                            skip_group_check=True)
                    if li < 2:
                        ht = _emit_elu(nc, sb, ot_ps[:], conv_b[li][:, 0:1],
                                       co, GROUP, f"c{li}_{g}",
                                       out_dtype=BF16)
                        bt_ps = pp.tile([P, NSUB * co], BF16,
                                        name=f"bt{li}_{g}", tag="psC")
                        for s in range(NSUB):
                            nc.tensor.transpose(
                                out=bt_ps[:, s * co:(s + 1) * co],
                                in_=ht[:, s * P:(s + 1) * P],
                                identity=identb[:co, :co])
                        bt_sb = sb.tile([P, NSUB * co], BF16,
                                        name=f"bts{li}_{g}", tag="bts")
                        nc.vector.tensor_copy(out=bt_sb[:], in_=bt_ps[:])
                        nc.sync.dma_start(
                            out=h_shard[li + 1][gsl, 0:co].rearrange(
                                "(s p) c -> p s c", p=P),
                            in_=bt_sb[:].rearrange("p (s c) -> p s c", c=co))
                    else:
                        ht = _emit_elu(nc, sb, ot_ps[:], conv_b[li][:, 0:1],
                                       co, GROUP, f"c{li}_{g}",
                                       out_dtype=F32R)
                        emit_tail(g, ht)
                if li < 2:
                    nc.gpsimd.collective_compute(
                        "AllGather", ALU.bypass, replica_groups=rg,
                        ins=[h_shard[li + 1][:]], outs=[h_full[li + 1][:]])

    nc.compile()
    return nc


def _prep_inputs(cfg: Cfg, x, indices, fc0_w, fc0_b, w1, b1, w2, b2, w3, b3,
                 fc1_w, fc1_b, fc2_w, fc2_b):
    """Shard + rearrange host inputs into per-core in_maps."""
    x = np.asarray(x, np.float32)
    idx = np.asarray(indices, np.int64)
    # node id -> padded table row
    rows = ((idx // cfg.shard) * cfg.shard_pad + idx % cfg.shard).astype(
        np.int32)
    quads = (rows // 4).astype(np.int16)
    offs = (rows % 4).astype(np.uint8)

    def conv_w_prep(w):
        # w [C_out, S*C] -> lhsT chunks [128, kch*C_out], bf16
        w = np.asarray(w, np.float32)
        co, sc = w.shape
        kch = sc // P
        return np.ascontiguousarray(
            w.T.reshape(kch, P, co).transpose(1, 0, 2).reshape(P, kch * co)
        ).astype(ml_dtypes.bfloat16)

    common = {
        "ident_in": np.eye(P, dtype=np.float32),
        "fc0_w_in": np.ascontiguousarray(np.asarray(fc0_w, np.float32).T),
        "b0_in": np.asarray(fc0_b, np.float32).reshape(16, 1),
        "w1_in": conv_w_prep(w1),
        "b1_in": np.asarray(b1, np.float32).reshape(-1, 1),
        "w2_in": conv_w_prep(w2),
        "b2_in": np.asarray(b2, np.float32).reshape(-1, 1),
        "w3_in": conv_w_prep(w3),
        "b3_in": np.asarray(b3, np.float32).reshape(-1, 1),
        "fc1_w_in": np.ascontiguousarray(np.asarray(fc1_w, np.float32).T),
        "fc1_b_in": np.ascontiguousarray(
            np.asarray(fc1_b, np.float32).reshape(2, P).T),
        "fc2_w_in": np.ascontiguousarray(
            np.asarray(fc2_w, np.float32).T.reshape(2, P, NUM_CLASSES)
            .transpose(1, 0, 2).reshape(P, 2 * NUM_CLASSES)),
        "fc2_b_in": np.tile(
            np.asarray(fc2_b, np.float32).reshape(1, NUM_CLASSES), (P, NSUB)),
    }

    ng = cfg.n_groups
    in_maps = []
    for c in range(N_CORES):
        lo = c * cfg.shard
        xs = np.zeros((cfg.shard_pad, IN_C), np.float32)
        xs[:cfg.shard] = x[lo:lo + cfg.shard]
        qc = np.zeros((cfg.shard_pad, SEQ), np.int16)
        qc[:cfg.shard] = quads[lo:lo + cfg.shard]
        oc = np.zeros((cfg.shard_pad, SEQ), np.uint8)
        oc[:cfg.shard] = offs[lo:lo + cfg.shard]

        # gather list position i = slot*128 + p, slot = s*16 + j,
        # node = g*512 + s*128 + p
        # qg[g, s, p, j] -> list[g, (s,j), p]
        qg = qc.reshape(ng, NSUB, P, SEQ)
        lists = qg.transpose(0, 1, 3, 2).reshape(ng, NIDX)   # [g, i]
        # wrapped [16, NIDX//16]: wrapped[i%16, i//16] = list[i], then
        # replicated 8x across the 128 partitions (one copy per Q7 core)
        wrapped = lists.reshape(ng, NIDX // 16, 16).transpose(0, 2, 1)
        gidx = np.tile(wrapped, (1, 8, 1))            # [ng, 128, 512]
        gidx = np.ascontiguousarray(
            gidx.transpose(1, 0, 2).reshape(P, ng * GROUP))

        og = oc.reshape(ng, NSUB, P, SEQ).transpose(0, 1, 3, 2) \
            .reshape(ng, 64, P)                               # [g, slot, p]
        msk = np.zeros((P, ng * 3 * 64), np.uint8)
        for t in (1, 2, 3):
            sel = (og == t).astype(np.uint8)                  # [g, slot, p]
            for g in range(ng):
                msk[:, g * 192 + (t - 1) * 64:g * 192 + t * 64] = \
                    sel[g].T                                  # [p, slot]
        in_maps.append({"x_in": xs, "gidx_in": gidx, "msk_in": msk, **common})
    return in_maps


_NC_CACHE = {}


def _get_nc(cfg: Cfg):
    key = cfg.shard_pad
    if key not in _NC_CACHE:
        _NC_CACHE[key] = build(cfg)
    return _NC_CACHE[key]


def kernel(**inputs) -> np.ndarray:
    cfg = FULL
    nc = _get_nc(cfg)
    in_maps = _prep_inputs(cfg, **inputs)
    res = run_bass_kernel_spmd(nc, in_maps, list(range(N_CORES)))
    out = np.concatenate(
        [res.results[c]["out"][:cfg.shard] for c in range(N_CORES)], axis=0)
    return out.astype(np.float32)


# revision 17
# speedup vs baseline: 2.7013x; 1.0234x over previous
"""Trainium2 Bass kernel for nn_Net_53386443489635 (spiral-conv GNN).

Data-parallel over nodes on 8 NeuronCores, v2 (batched quad gathers):
  - each core computes a 12500-node shard (padded to 12800) of every layer
  - h0/h1/h2 feature tables are bf16, stored quad-packed in DRAM (4 nodes
    per 256B/256B/512B row) and AllGathered between conv layers
  - neighbor gathers run as ONE dma_gather per 512-node group (8192 int16
    quad indices, positional layout chosen host-side so the output lands
    node-major), then a 4-way predicated select resolves the within-quad
    offset using host-precomputed masks
  - selected bf16 tiles are PE-transposed to feature-major, conv matmuls run
    bf16 x bf16 with fp32 PSUM accumulation
  - ELU = relu(y) + exp(y - relu(y)) - 1 split across DVE and ACT
  - conv3 -> fc1 -> fc2 run fp32r; fc2 emits node-major logits and
    log_softmax runs along the free axis
"""

import sys

for _p in ("/opt/trn_rl_repo", "/root/.axon_site/_ro/trn_rl_repo"):
    if _p not in sys.path:
        sys.path.append(_p)

import numpy as np
import ml_dtypes

import concourse.bass as bass
import concourse.bacc as bacc
import concourse.mybir as mybir
import concourse.tile as tile
from concourse.bass_utils import run_bass_kernel_spmd

F32 = mybir.dt.float32
F32R = mybir.dt.float32r
BF16 = mybir.dt.bfloat16
I16 = mybir.dt.int16
U8 = mybir.dt.uint8
ALU = mybir.AluOpType
AF = mybir.ActivationFunctionType
AX = mybir.AxisListType

N_CORES = 8
P = 128
SEQ = 16
IN_C = 3
NUM_CLASSES = 12
GROUP = 512          # nodes per group (4 sub-tiles of 128)
NSUB = GROUP // P    # 4
NIDX = GROUP * SEQ   # 8192 gathers per group

# conv layer l: (S*C_in, C_in, C_out, table pitch in bf16 elems per node)
CONV_DIMS = [(SEQ * 16, 16, 32, 32), (SEQ * 32, 32, 64, 32),
             (SEQ * 64, 64, 128, 64)]
PITCHES = [32, 32, 64]   # h0, h1, h2 table pitches (>= real ch, 64B-mult/4)


class Cfg:
    def __init__(self, n_nodes=100000, shard=12500, shard_pad=12800):
        assert shard_pad % GROUP == 0
        self.n_nodes = n_nodes
        self.shard = shard
        self.shard_pad = shard_pad
        self.n_groups = shard_pad // GROUP
        self.table = N_CORES * shard_pad
        self.n_quads = self.table // 4


FULL = Cfg()
MINI = Cfg(n_nodes=4000, shard=500, shard_pad=512)


def _emit_elu(nc, sb, psum_in, bias_col, c_out, width, tag, out_dtype=F32):
    """elu(psum_in + bias) -> SBUF tile [c_out, width]. 3 DVE + 2 ACT ops."""
    y = sb.tile([c_out, width], F32, name=f"y_{tag}", tag="elu_y")
    nc.vector.tensor_scalar(out=y[:], in0=psum_in, scalar1=bias_col,
                            scalar2=None, op0=ALU.add)
    r = sb.tile([c_out, width], F32, name=f"r_{tag}", tag="elu_r")
    nc.scalar.activation(out=r[:], in_=y[:], func=AF.Relu)
    n = sb.tile([c_out, width], F32, name=f"n_{tag}", tag="elu_n")
    nc.vector.tensor_tensor(out=n[:], in0=y[:], in1=r[:], op=ALU.subtract)
    e = sb.tile([c_out, width], F32, name=f"e_{tag}", tag="elu_e")
    nc.scalar.activation(out=e[:], in_=n[:], func=AF.Exp)
    h = sb.tile([c_out, width], out_dtype, name=f"h_{tag}", tag=f"h_{tag[:2]}")
    nc.vector.scalar_tensor_tensor(out=h[:], in0=e[:], scalar=-1.0, in1=r[:],
                                   op0=ALU.add, op1=ALU.add)
    return h


def build(cfg: Cfg):
    nc = bacc.Bacc("TRN2", target_bir_lowering=False, debug=False,
                   enable_asserts=True, num_devices=N_CORES,
                   num_swdge_queues=4)

    sp = cfg.shard_pad
    ng = cfg.n_groups

    # ---- I/O ----
    x_in = nc.dram_tensor("x_in", [sp, IN_C], F32, kind="ExternalInput")
    gidx_in = nc.dram_tensor("gidx_in", [P, ng * GROUP], I16,
                             kind="ExternalInput")
    msk_in = nc.dram_tensor("msk_in", [P, ng * 3 * 64], U8,
                            kind="ExternalInput")
    ident_in = nc.dram_tensor("ident_in", [P, P], F32, kind="ExternalInput")
    fc0_w_in = nc.dram_tensor("fc0_w_in", [IN_C, 16], F32,
                              kind="ExternalInput")
    b0_in = nc.dram_tensor("b0_in", [16, 1], F32, kind="ExternalInput")
    w_ins, b_ins = [], []
    for li, (sc, ci, co, _pt) in enumerate(CONV_DIMS):
        w_ins.append(nc.dram_tensor(f"w{li + 1}_in", [P, (sc // P) * co],
                                    BF16, kind="ExternalInput"))
        b_ins.append(nc.dram_tensor(f"b{li + 1}_in", [co, 1], F32,
                                    kind="ExternalInput"))
    fc1_w_in = nc.dram_tensor("fc1_w_in", [P, 256], F32, kind="ExternalInput")
    fc1_b_in = nc.dram_tensor("fc1_b_in", [P, 2], F32, kind="ExternalInput")
    fc2_w_in = nc.dram_tensor("fc2_w_in", [P, 2 * NUM_CLASSES], F32,
                              kind="ExternalInput")
    fc2_b_in = nc.dram_tensor("fc2_b_in", [P, NSUB * NUM_CLASSES], F32,
                              kind="ExternalInput")
    out_dram = nc.dram_tensor("out", [sp, NUM_CLASSES], F32,
                              kind="ExternalOutput")

    # ---- internal DRAM: bf16 quad-packed tables ----
    h_shard = [nc.dram_tensor(f"h{i}_shard", [sp, PITCHES[i]], BF16)
               for i in range(3)]
    h_full = [nc.dram_tensor(f"h{i}_full", [cfg.n_quads, 4 * PITCHES[i]],
                             BF16, addr_space="Shared")
              for i in range(3)]

    rg = [list(range(N_CORES))]

    with tile.TileContext(nc) as tc:
        with (
            tc.tile_pool(name="sbuf", bufs=2) as sb,
            tc.tile_pool(name="wpool", bufs=1) as wp,
            tc.tile_pool(name="psum", bufs=2, space="PSUM") as pp,
        ):
            # ---- resident tiles ----
            gidx_sb = wp.tile([P, ng * GROUP], I16, name="gidx_sb")
            nc.sync.dma_start(out=gidx_sb[:], in_=gidx_in[:])
            msk_sb = wp.tile([P, ng * 3 * 64], U8, name="msk_sb")
            nc.sync.dma_start(out=msk_sb[:], in_=msk_in[:])
            ident = wp.tile([P, P], F32, name="ident")
            nc.sync.dma_start(out=ident[:], in_=ident_in[:])
            identb = wp.tile([P, P], BF16, name="identb")
            nc.vector.tensor_copy(out=identb[:], in_=ident[:])

            def load_r(name, shape, src):
                """DMA fp32 -> SBUF, round once into an F32R tile."""
                t = wp.tile(shape, F32, name=f"{name}_raw")
                nc.sync.dma_start(out=t[:], in_=src[:])
                tr = wp.tile(shape, F32R, name=name)
                nc.vector.tensor_copy(out=tr[:], in_=t[:])
                return tr

            fc0_w = load_r("fc0_w", [IN_C, 16], fc0_w_in)
            b0 = wp.tile([16, 1], F32, name="b0")
            nc.sync.dma_start(out=b0[:], in_=b0_in[:])
            conv_w, conv_b = [], []
            for li, (sc, ci, co, _pt) in enumerate(CONV_DIMS):
                w = wp.tile([P, (sc // P) * co], BF16, name=f"w{li + 1}")
                nc.sync.dma_start(out=w[:], in_=w_ins[li][:])
                conv_w.append(w)
                b = wp.tile([co, 1], F32, name=f"bb{li + 1}")
                nc.sync.dma_start(out=b[:], in_=b_ins[li][:])
                conv_b.append(b)
            fc1_w = load_r("fc1_w", [P, 256], fc1_w_in)
            fc1_b = wp.tile([P, 2], F32, name="fc1_b")
            nc.sync.dma_start(out=fc1_b[:], in_=fc1_b_in[:])
            fc2_w = load_r("fc2_w", [P, 2 * NUM_CLASSES], fc2_w_in)
            fc2_b = wp.tile([P, NSUB * NUM_CLASSES], F32, name="fc2_b")
            nc.sync.dma_start(out=fc2_b[:], in_=fc2_b_in[:])

            # ---- fc0: x -> h0_shard (node-major bf16) ----
            for g in range(ng):
                gsl = slice(g * GROUP, (g + 1) * GROUP)
                x_sb = sb.tile([P, NSUB * IN_C], F32, name=f"x_{g}", tag="x")
                nc.sync.dma_start(
                    out=x_sb[:].rearrange("p (s c) -> p s c", c=IN_C),
                    in_=x_in[gsl, :].rearrange("(s p) c -> p s c", p=P))
                xt_ps = pp.tile([IN_C, GROUP], F32, name=f"xtp_{g}", tag="psA")
                for s in range(NSUB):
                    nc.tensor.transpose(
                        out=xt_ps[:, s * P:(s + 1) * P],
                        in_=x_sb[:, s * IN_C:(s + 1) * IN_C],
                        identity=ident[:])
                xt_sb = sb.tile([IN_C, GROUP], F32R, name=f"xts_{g}",
                                tag="xts")
                nc.vector.tensor_copy(out=xt_sb[:], in_=xt_ps[:])
                h0t_ps = pp.tile([16, GROUP], F32, name=f"h0tp_{g}", tag="psB")
                nc.tensor.matmul(out=h0t_ps[:], lhsT=fc0_w[:], rhs=xt_sb[:],
                                 start=True, stop=True, skip_group_check=True)
                h0t = _emit_elu(nc, sb, h0t_ps[:], b0[:, 0:1], 16, GROUP,
                                f"f0_{g}", out_dtype=BF16)
                bt_ps = pp.tile([P, NSUB * 16], BF16, name=f"h0bt_{g}",
                                tag="psC")
                for s in range(NSUB):
                    nc.tensor.transpose(
                        out=bt_ps[:, s * 16:(s + 1) * 16],
                        in_=h0t[:, s * P:(s + 1) * P],
                        identity=identb[:16, :16])
                bt_sb = sb.tile([P, NSUB * 16], BF16, name=f"h0bs_{g}",
                                tag="bts")
                nc.vector.tensor_copy(out=bt_sb[:], in_=bt_ps[:])
                nc.sync.dma_start(
                    out=h_shard[0][gsl, 0:16].rearrange("(s p) c -> p s c",
                                                        p=P),
                    in_=bt_sb[:].rearrange("p (s c) -> p s c", c=16))

            nc.gpsimd.collective_compute(
                "AllGather", ALU.bypass, replica_groups=rg,
                ins=[h_shard[0][:]], outs=[h_full[0][:]])

            # ---- fc1 + fc2 + log_softmax tail (runs per conv3 group) ----
            def emit_tail(g, h3t):
                gsl = slice(g * GROUP, (g + 1) * GROUP)
                h4t = []
                for half in range(2):
                    h4_ps = pp.tile([P, GROUP], F32, name=f"h4p_{g}_{half}",
                                    tag="psB")
                    nc.tensor.matmul(
                        out=h4_ps[:], lhsT=fc1_w[:, half * P:(half + 1) * P],
                        rhs=h3t[:], start=True, stop=True,
                        skip_group_check=True)
                    h4t.append(_emit_elu(nc, sb, h4_ps[:],
                                         fc1_b[:, half:half + 1], P, GROUP,
                                         f"f{half}1_{g}", out_dtype=F32R))
                nclw = NUM_CLASSES
                lg_ps = pp.tile([P, NSUB * nclw], F32, name=f"lg_{g}",
                                tag="psD", bufs=1)
                for s in range(NSUB):
                    for half in range(2):
                        nc.tensor.matmul(
                            out=lg_ps[:, s * nclw:(s + 1) * nclw],
                            lhsT=h4t[half][:, s * P:(s + 1) * P],
                            rhs=fc2_w[:, half * nclw:(half + 1) * nclw],
                            start=(half == 0), stop=(half == 1),
                            skip_group_check=True)
                lg_sb = sb.tile([P, NSUB * nclw], F32, name=f"lgs_{g}",
                                tag="lgs")
                nc.vector.tensor_tensor(out=lg_sb[:], in0=lg_ps[:],
                                        in1=fc2_b[:], op=ALU.add)
                lg3 = lg_sb[:].rearrange("p (s c) -> p s c", c=nclw)
                mx = sb.tile([P, NSUB], F32, name=f"mx_{g}", tag="mx")
                nc.vector.tensor_reduce(
                    out=mx[:].rearrange("p (s o) -> p s o", o=1),
                    in_=lg3, axis=AX.X, op=ALU.max)
                sh_sb = sb.tile([P, NSUB * nclw], F32, name=f"sh_{g}",
                                tag="sh")
                nc.vector.tensor_tensor(
                    out=sh_sb[:].rearrange("p (s c) -> p s c", c=nclw),
                    in0=lg3,
                    in1=mx[:].rearrange("p (s o) -> p s o", o=1).to_broadcast(
                        [P, NSUB, nclw]),
                    op=ALU.subtract)
                ex_sb = sb.tile([P, NSUB * nclw], F32, name=f"ex_{g}",
                                tag="ex")
                nc.scalar.activation(out=ex_sb[:], in_=sh_sb[:], func=AF.Exp)
                sm = sb.tile([P, NSUB], F32, name=f"sm_{g}", tag="sm")
                nc.vector.tensor_reduce(
                    out=sm[:].rearrange("p (s o) -> p s o", o=1),
                    in_=ex_sb[:].rearrange("p (s c) -> p s c", c=nclw),
                    axis=AX.X, op=ALU.add)
                ls = sb.tile([P, NSUB], F32, name=f"ls_{g}", tag="ls")
                nc.scalar.activation(out=ls[:], in_=sm[:], func=AF.Ln)
                res = sb.tile([P, NSUB * nclw], F32, name=f"res_{g}",
                              tag="res")
                nc.vector.tensor_tensor(
                    out=res[:].rearrange("p (s c) -> p s c", c=nclw),
                    in0=sh_sb[:].rearrange("p (s c) -> p s c", c=nclw),
                    in1=ls[:].rearrange("p (s o) -> p s o", o=1).to_broadcast(
                        [P, NSUB, nclw]),
                    op=ALU.subtract)
                nc.sync.dma_start(
                    out=out_dram[gsl, :].rearrange("(s p) c -> p s c", p=P),
                    in_=res[:].rearrange("p (s c) -> p s c", c=nclw))

            # ---- conv layers ----
            for li, (sc, ci, co, pitch) in enumerate(CONV_DIMS):
                kch = sc // P
                qelem = 4 * pitch
                src = h_full[li]
                for g in range(ng):
                    gsl = slice(g * GROUP, (g + 1) * GROUP)
                    # one batched quad gather for the whole group
                    qt = sb.tile([P, 64 * qelem], BF16, name=f"qt{li}_{g}",
                                 tag="qt")
                    for q in range(4):
                        nc.gpsimd.dma_gather(
                            out_ap=qt[:, q * 16 * qelem:(q + 1) * 16 * qelem]
                            .rearrange("p (s e) -> p s e", e=qelem),
                            in_ap=src[:],
                            idxs_ap=gidx_sb[:, g * GROUP + q * 128:
                                            g * GROUP + (q + 1) * 128],
                            num_idxs=NIDX // 4,
                            num_idxs_reg=NIDX // 4,
                            elem_size=qelem,
                            single_packet=False,
                            queue_num=q)
                    qv = qt[:].rearrange("p (s o c) -> p s o c", o=4, c=pitch)
                    # 4-way within-quad select into compact node-major tile
                    cp = sb.tile([P, 64 * ci], BF16, name=f"cp{li}_{g}",
                                 tag="cp")
                    cpv = cp[:].rearrange("p (s c) -> p s c", c=ci)
                    nc.vector.tensor_copy(out=cpv, in_=qv[:, :, 0, 0:ci])
                    for t in (1, 2, 3):
                        mbase = g * 192 + (t - 1) * 64
                        m = msk_sb[:, mbase:mbase + 64].rearrange(
                            "p (s u) -> p s u", u=1).to_broadcast([P, 64, ci])
                        nc.vector.copy_predicated(out=cpv, mask=m,
                                                  data=qv[:, :, t, 0:ci])
                    # feature-major transposes + matmul
                    ot_ps = pp.tile([co, GROUP], F32, name=f"ot{li}_{g}",
                                    tag="psB")
                    for k in range(kch):
                        gtp = pp.tile([P, GROUP], BF16, name=f"gtp{li}_{g}_{k}",
                                      tag="psA")
                        for s in range(NSUB):
                            nc.tensor.transpose(
                                out=gtp[:, s * P:(s + 1) * P],
                                in_=cp[:, s * sc + k * P:s * sc + (k + 1) * P],
                                identity=identb[:])
                        gts = sb.tile([P, GROUP], BF16, name=f"gts{li}_{g}_{k}",
                                      tag="gts")
                        nc.scalar.copy(out=gts[:], in_=gtp[:])
                        nc.tensor.matmul(
                            out=ot_ps[:],
                            lhsT=conv_w[li][:, k * co:(k + 1) * co],
                            rhs=gts[:], start=(k == 0), stop=(k == kch - 1),
                            skip_group_check=True)
                    if li < 2:
                        ht = _emit_elu(nc, sb, ot_ps[:], conv_b[li][:, 0:1],
                                       co, GROUP, f"c{li}_{g}",
                                       out_dtype=BF16)
                        bt_ps = pp.tile([P, NSUB * co], BF16,
                                        name=f"bt{li}_{g}", tag="psC")
                        for s in range(NSUB):
                            nc.tensor.transpose(
                                out=bt_ps[:, s * co:(s + 1) * co],
                                in_=ht[:, s * P:(s + 1) * P],
                                identity=identb[:co, :co])
                        bt_sb = sb.tile([P, NSUB * co], BF16,
                                        name=f"bts{li}_{g}", tag="bts")
                        nc.vector.tensor_copy(out=bt_sb[:], in_=bt_ps[:])
                        nc.sync.dma_start(
                            out=h_shard[li + 1][gsl, 0:co].rearrange(
                                "(s p) c -> p s c", p=P),
                            in_=bt_sb[:].rearrange("p (s c) -> p s c", c=co))
                    else:
                        ht = _emit_elu(nc, sb, ot_ps[:], conv_b[li][:, 0:1],
                                       co, GROUP, f"c{li}_{g}",
                                       out_dtype=F32R)
                        emit_tail(g, ht)
                if li < 2:
                    nc.gpsimd.collective_compute(
                        "AllGather", ALU.bypass, replica_groups=rg,
                        ins=[h_shard[li + 1][:]], outs=[h_full[li + 1][:]])

    nc.compile()
    return nc


def _prep_inputs(cfg: Cfg, x, indices, fc0_w, fc0_b, w1, b1, w2, b2, w3, b3,
                 fc1_w, fc1_b, fc2_w, fc2_b):
    """Shard + rearrange host inputs into per-core in_maps."""
    x = np.asarray(x, np.float32)
    idx = np.asarray(indices, np.int64)
    # node id -> padded table row
    rows = ((idx // cfg.shard) * cfg.shard_pad + idx % cfg.shard).astype(
        np.int32)
    quads = (rows // 4).astype(np.int16)
    offs = (rows % 4).astype(np.uint8)

    def conv_w_prep(w):
        # w [C_out, S*C] -> lhsT chunks [128, kch*C_out], bf16
        w = np.asarray(w, np.float32)
        co, sc = w.shape
        kch = sc // P
        return np.ascontiguousarray(
            w.T.reshape(kch, P, co).transpose(1, 0, 2).reshape(P, kch * co)
        ).astype(ml_dtypes.bfloat16)

    common = {
        "ident_in": np.eye(P, dtype=np.float32),
        "fc0_w_in": np.ascontiguousarray(np.asarray(fc0_w, np.float32).T),
        "b0_in": np.asarray(fc0_b, np.float32).reshape(16, 1),
        "w1_in": conv_w_prep(w1),
        "b1_in": np.asarray(b1, np.float32).reshape(-1, 1),
        "w2_in": conv_w_prep(w2),
        "b2_in": np.asarray(b2, np.float32).reshape(-1, 1),
        "w3_in": conv_w_prep(w3),
        "b3_in": np.asarray(b3, np.float32).reshape(-1, 1),
        "fc1_w_in": np.ascontiguousarray(np.asarray(fc1_w, np.float32).T),
        "fc1_b_in": np.ascontiguousarray(
            np.asarray(fc1_b, np.float32).reshape(2, P).T),
        "fc2_w_in": np.ascontiguousarray(
            np.asarray(fc2_w, np.float32).T.reshape(2, P, NUM_CLASSES)
            .transpose(1, 0, 2).reshape(P, 2 * NUM_CLASSES)),
        "fc2_b_in": np.tile(
            np.asarray(fc2_b, np.float32).reshape(1, NUM_CLASSES), (P, NSUB)),
    }

    ng = cfg.n_groups
    in_maps = []
    for c in range(N_CORES):
        lo = c * cfg.shard
        xs = np.zeros((cfg.shard_pad, IN_C), np.float32)
        xs[:cfg.shard] = x[lo:lo + cfg.shard]
        qc = np.zeros((cfg.shard_pad, SEQ), np.int16)
        qc[:cfg.shard] = quads[lo:lo + cfg.shard]
        oc = np.zeros((cfg.shard_pad, SEQ), np.uint8)
        oc[:cfg.shard] = offs[lo:lo + cfg.shard]

        # gather list position i = slot*128 + p, slot = s*16 + j,
        # node = g*512 + s*128 + p
        # qg[g, s, p, j] -> list[g, (s,j), p]
        qg = qc.reshape(ng, NSUB, P, SEQ)
        lists = qg.transpose(0, 1, 3, 2).reshape(ng, NIDX)   # [g, i]
        # wrapped [16, NIDX//16]: wrapped[i%16, i//16] = list[i], then
        # replicated 8x across the 128 partitions (one copy per Q7 core)
        wrapped = lists.reshape(ng, NIDX // 16, 16).transpose(0, 2, 1)
        gidx = np.tile(wrapped, (1, 8, 1))            # [ng, 128, 512]
        gidx = np.ascontiguousarray(
            gidx.transpose(1, 0, 2).reshape(P, ng * GROUP))

        og = oc.reshape(ng, NSUB, P, SEQ).transpose(0, 1, 3, 2) \
            .reshape(ng, 64, P)                               # [g, slot, p]
        msk = np.zeros((P, ng * 3 * 64), np.uint8)
        for t in (1, 2, 3):
            sel = (og == t).astype(np.uint8)                  # [g, slot, p]
            for g in range(ng):
                msk[:, g * 192 + (t - 1) * 64:g * 192 + t * 64] = \
                    sel[g].T                                  # [p, slot]
        in_maps.append({"x_in": xs, "gidx_in": gidx, "msk_in": msk, **common})
    return in_maps


_NC_CACHE = {}


def _get_nc(cfg: Cfg):
    key = cfg.shard_pad
    if key not in _NC_CACHE:
        _NC_CACHE[key] = build(cfg)
    return _NC_CACHE[key]


def kernel(**inputs) -> np.ndarray:
    cfg = FULL
    nc = _get_nc(cfg)
    in_maps = _prep_inputs(cfg, **inputs)
    res = run_bass_kernel_spmd(nc, in_maps, list(range(N_CORES)))
    out = np.concatenate(
        [res.results[c]["out"][:cfg.shard] for c in range(N_CORES)], axis=0)
    return out.astype(np.float32)
